# revision 1
# baseline (speedup 1.0000x reference)
import sys

if "/opt/trn_rl_repo" not in sys.path:
    sys.path.insert(0, "/opt/trn_rl_repo")

import numpy as np

NCORES = 8
B = 65536
NPC = B // NCORES  # 8192 images per core
G = 8              # image-tiles (of 128) per group
NGROUPS = NPC // (128 * G)
MAGIC = 12582912.0  # 1.5 * 2**23: (v+M)-M == round-to-nearest-even, |v| < 2**22
AF = 128.0 / 127.5

_cache = {}


def _build(wq9, ndve=5):
    """wq9: tuple of 9 floats, quantized conv taps in {0,+-0.5}, row-major.
    ndve: how many of the non-center taps run on DVE (rest on Pool)."""
    from contextlib import ExitStack

    import concourse.tile as tile
    from concourse import bacc, mybir

    f32 = mybir.dt.float32
    f16 = mybir.dt.float16
    Alu = mybir.AluOpType
    Act = mybir.ActivationFunctionType

    nc = bacc.Bacc("TRN2", target_bir_lowering=False, debug=False,
                   num_devices=NCORES)
    x = nc.dram_tensor("x", [NPC, 576], f32, kind="ExternalInput").ap()
    wfc = nc.dram_tensor("wfc", [256, 10], f16, kind="ExternalInput").ap()
    out = nc.dram_tensor("out", [10, NPC], f32, kind="ExternalOutput").ap()

    with tile.TileContext(nc) as tc, ExitStack() as ctx:
        consts = ctx.enter_context(tc.tile_pool(name="consts", bufs=1))
        w1 = consts.tile([128, 10], f16)
        w2 = consts.tile([128, 10], f16)
        nc.sync.dma_start(w1[:], wfc[0:128, :])
        nc.sync.dma_start(w2[:], wfc[128:256, :])

        xpool = ctx.enter_context(tc.tile_pool(name="xp", bufs=2))
        hpool = ctx.enter_context(tc.tile_pool(name="hp", bufs=2))
        yapool = ctx.enter_context(tc.tile_pool(name="yap", bufs=2))
        ybpool = ctx.enter_context(tc.tile_pool(name="ybp", bufs=2))
        ppool = ctx.enter_context(tc.tile_pool(name="pp", bufs=2))
        apool = ctx.enter_context(tc.tile_pool(name="ap", bufs=2))
        tpool = ctx.enter_context(tc.tile_pool(name="tp", bufs=4))
        spool = ctx.enter_context(tc.tile_pool(name="sp", bufs=2))
        po = ctx.enter_context(tc.tile_pool(name="po", bufs=2, space="PSUM"))

        xv_dram = x.rearrange("(g a p) f -> g p a f", p=128, a=G)

        # taps scaled x2 so they land in {0,+-1}: pure add/subtract on A/2
        cen = 2.0 * wq9[4]
        taps = [(dr, dc, 2.0 * wq9[(dr + 1) * 3 + (dc + 1)])
                for dr in (-1, 0, 1) for dc in (-1, 0, 1)
                if not (dr == 0 and dc == 0)
                and wq9[(dr + 1) * 3 + (dc + 1)] != 0.0]
        dve_taps = taps[:ndve]
        pool_taps = taps[ndve:]

        R = G * 24
        for g in range(NGROUPS):
            xt = xpool.tile([128, G * 576], f32)
            nc.sync.dma_start(xt[:].rearrange("p (a f) -> p a f", a=G),
                              xv_dram[g])
            # quantize: A = clamp(round(x*AF - 128), -127, 127); xh = A/2 fp16
            nc.scalar.activation(xt[:], xt[:], Act.Copy,
                                 bias=MAGIC - 128.0, scale=AF)
            nc.vector.tensor_scalar(xt[:], xt[:], MAGIC, -127.0,
                                    Alu.subtract, Alu.max)
            xh = hpool.tile([128, G * 576], f16)
            nc.gpsimd.tensor_scalar(xh[:], xt[:], 127.0, 0.5,
                                    Alu.min, Alu.mult)

            # 3x3 SAME conv (x128 domain) as shifted +-xh adds, split across
            # two accumulators so DVE and Pool run independent chains.
            ya = yapool.tile([128, G * 576], f16)
            yb = ybpool.tile([128, G * 576], f16)
            nc.scalar.activation(ya[:], xh[:], Act.Copy, bias=0.0, scale=cen)
            nc.gpsimd.tensor_scalar_mul(yb[:], xh[:], 0.0)

            xr = xh[:].rearrange("p (r w) -> p r w", w=24)
            xa = xh[:].rearrange("p (a f) -> p a f", a=G)
            for eng, yt, tlist in ((nc.vector, ya, dve_taps),
                                   (nc.gpsimd, yb, pool_taps)):
                yr = yt[:].rearrange("p (r w) -> p r w", w=24)
                yv = yt[:].rearrange("p (a f) -> p a f", a=G)
                for dr, dc, s in tlist:
                    op = Alu.add if s > 0 else Alu.subtract
                    cop = Alu.subtract if s > 0 else Alu.add
                    co0, co1 = max(0, -dc), 24 - max(0, dc)
                    if dr == 0:
                        eng.tensor_tensor(yr[:, :, co0:co1],
                                          yr[:, :, co0:co1],
                                          xr[:, :, co0 + dc:co1 + dc], op)
                        continue
                    r0, r1 = max(0, -dr), R - max(0, dr)
                    eng.tensor_tensor(
                        yr[:, r0:r1, co0:co1], yr[:, r0:r1, co0:co1],
                        xr[:, r0 + dr:r1 + dr, co0 + dc:co1 + dc], op)
                    # cancel cross-image leakage on the G-1 boundary rows
                    if dr == 1:
                        ysl = yv[:, 0:G - 1, 23 * 24 + co0:23 * 24 + co1]
                        xsl = xa[:, 1:G, co0 + dc:co1 + dc]
                    else:
                        ysl = yv[:, 1:G, co0:co1]
                        xsl = xa[:, 0:G - 1,
                                 23 * 24 + co0 + dc:23 * 24 + co1 + dc]
                    eng.tensor_tensor(ysl, ysl, xsl, cop)

            nc.vector.tensor_tensor(ya[:], ya[:], yb[:], Alu.add)

            # maxpool 2x2 -> 12x12 interior (pad ring pools to zero, dropped)
            p1 = ppool.tile([128, G * 288], f16)
            yv4 = ya[:].rearrange("p (r t w) -> p r t w", t=2, w=24)
            p1r = p1[:].rearrange("p (r w) -> p r w", w=24)
            nc.vector.tensor_tensor(p1r, yv4[:, :, 0, :], yv4[:, :, 1, :],
                                    Alu.max)
            act = apool.tile([128, G * 144], f16)
            p1v4 = p1[:].rearrange("p (r w t) -> p r w t", w=12, t=2)
            actr = act[:].rearrange("p (r w) -> p r w", w=12)
            nc.vector.tensor_tensor(actr, p1v4[:, :, :, 0], p1v4[:, :, :, 1],
                                    Alu.max)
            # relu + clip 127 + round (fp16 magic 1536 = 1.5*2**10)
            nc.vector.tensor_scalar(act[:], act[:], 0.0, 127.0,
                                    Alu.max, Alu.min)
            nc.vector.tensor_scalar(act[:], act[:], 1536.0, 1536.0,
                                    Alu.add, Alu.subtract)

            # FC: out^T[o, b] = sum_k W[k, o] actT[k, b], K=144 as two
            # 128-partition matmuls: actT of feats 0:128 vs W_A, and of
            # feats 16:144 vs W_B (zeros except rows 112:128 = feats 128:144)
            for h in range(2):
                aT1 = tpool.tile([128, 512], f16)
                aT2 = tpool.tile([128, 512], f16)
                for j in range(4):
                    a = h * 4 + j
                    nc.sync.dma_start_transpose(
                        aT1[:, j * 128:(j + 1) * 128],
                        act[:, a * 144:a * 144 + 128])
                    nc.sync.dma_start_transpose(
                        aT2[:, j * 128:(j + 1) * 128],
                        act[:, a * 144 + 16:a * 144 + 144])
                pOT = po.tile([10, 512], f32)
                nc.tensor.matmul(pOT[:], w1[:], aT1[:], start=True, stop=False)
                nc.tensor.matmul(pOT[:], w2[:], aT2[:], start=False, stop=True)
                soT = spool.tile([10, 512], f32)
                nc.scalar.copy(soT[:], pOT[:])
                nc.sync.dma_start(
                    out[:, g * 1024 + h * 512:g * 1024 + (h + 1) * 512],
                    soT[:])

    nc.compile()
    return nc


def _prep(conv_w, fc_w):
    # replicate reference weight quantization exactly (all steps exact in f32)
    cw = np.asarray(conv_w, np.float32).reshape(3, 3)
    wq = (np.round(np.clip(cw, -0.5, 0.5) * 2.0) / 2.0).astype(np.float32)
    fw = np.asarray(fc_w, np.float32)
    wfq = (np.round(np.clip(fw, -0.5, 0.5) * 2.0) / 2.0 / 8.0).astype(np.float32)
    # FC sees act128/128; fold the /128 into W (values k/2048, exact fp16).
    # Rows 0:128 = feats 0:128 (W_A); rows 240:256 = feats 128:144 placed at
    # partition 112+ of W_B to match the feats-16:144 transposed tile.
    Wdev = np.zeros((256, 10), np.float32)
    for i in range(12):
        for j in range(12):
            k = i * 12 + j
            r = k if k < 128 else k + 112
            Wdev[r, :] = wfq[:, (i + 1) * 14 + (j + 1)] / 128.0
    return tuple(float(v) for v in wq.flatten()), Wdev.astype(np.float16)


def _get_program(wq9, ndve=5):
    key = (wq9, ndve)
    nc = _cache.get(key)
    if nc is None:
        nc = _build(wq9, ndve)
        _cache[key] = nc
    return nc


def _make_in_maps(x2d, Wdev):
    return [{"x": np.ascontiguousarray(x2d[c * NPC:(c + 1) * NPC]),
             "wfc": Wdev} for c in range(NCORES)]


def run(x, conv_w, fc_w, trace=False, **kw):
    from concourse.bass_utils import run_bass_kernel_spmd

    x2d = np.ascontiguousarray(
        np.asarray(x, np.float32).reshape(B, 576))
    wq9, Wdev = _prep(conv_w, fc_w)
    nc = _get_program(wq9)
    res = run_bass_kernel_spmd(nc, _make_in_maps(x2d, Wdev),
                               core_ids=list(range(NCORES)),
                               trace=trace, **kw)
    out = np.concatenate([np.asarray(r["out"]).T for r in res.results], axis=0)
    return np.ascontiguousarray(out.astype(np.float32)), res


def kernel(x, conv_w, fc_w):
    out, _ = run(x, conv_w, fc_w, trace=False)
    return out



# revision 11
# speedup vs baseline: 5.3378x; 5.3378x over previous
import sys

if "/opt/trn_rl_repo" not in sys.path:
    sys.path.insert(0, "/opt/trn_rl_repo")

import numpy as np

NCORES = 8
B = 65536
NPC = B // NCORES  # 8192 images per core
G = 8              # image-tiles (of 128) per group
NGROUPS = NPC // (128 * G)
AF = 128.0 / 127.5
IS = 626           # image block: 25 rows x 25 cols + 1 spare (even stride)
LEAD = 32          # leading pad cells (>= 26 so dr=-1,dc=-1 reads stay in-tile)
TW = LEAD + G * IS + 32  # f16 cells per partition for padded tiles

_cache = {}


def _build(wq9):
    """wq9: tuple of 9 floats, quantized conv taps in {0,+-0.5}, row-major.

    Pipeline per group (128 partitions x G images):
      scalar: t=f16(AF*x+1408) [=A+1536, f16 convert rounds to int]
              v=Relu(t-1409) [=max(A+127,0)]  xh2=Relu(254-v) [=127-clip(A)]
      DVE:    y' = sum_i sigma_i * shift(xh2), sigma=-sign(w): 7 ops via
              scalar_tensor_tensor + tensor_tensor on a 25x25-padded layout
              (pads hold 127 == "A=0"), then 2x2 maxpool (2 ops) and the
              final clamp; all values exact integers in f16.
      scalar: u=f16(0.5*pool + (1536-63.5*C))  [= round(128*P)+1536]
      PE:     FC as two matmuls per 512-batch; +1536 bias folded into the
              PSUM->SBUF copy via a per-partition bias AP.
    """
    from contextlib import ExitStack

    import concourse.tile as tile
    from concourse import bacc, mybir

    f32 = mybir.dt.float32
    f16 = mybir.dt.float16
    Alu = mybir.AluOpType
    Act = mybir.ActivationFunctionType

    nc = bacc.Bacc("TRN2", target_bir_lowering=False, debug=False,
                   num_devices=NCORES)

    # non-Copy activation biases need pre-registered const APs
    for cval in (-1409.0, 254.0):
        ctensor = nc.alloc_sbuf_tensor(f"constb-{cval}", [128, 1], f32)
        nc.gpsimd.memset(ctensor.ap(), cval)
        nc.const_aps.aps[(f32, cval)] = ctensor.ap()
    nc.all_engine_barrier()

    x = nc.dram_tensor("x", [NPC, 576], f32, kind="ExternalInput").ap()
    wfc = nc.dram_tensor("wfc", [256, 10], f16, kind="ExternalInput").ap()
    fcb = nc.dram_tensor("fcb", [10, 1], f32, kind="ExternalInput").ap()
    out = nc.dram_tensor("out", [10, NPC], f32, kind="ExternalOutput").ap()

    # taps: (dr, dc, sigma) with sigma = -sign(w); order interleaves signs to
    # keep partial sums exact in f16 (|partial| <= 2048)
    taps = [(dr, dc, -1.0 if wq9[(dr + 1) * 3 + (dc + 1)] > 0 else 1.0)
            for dr in (-1, 0, 1) for dc in (-1, 0, 1)
            if wq9[(dr + 1) * 3 + (dc + 1)] != 0.0]
    taps.sort(key=lambda t: t[2])
    order, pos, neg = [], [t for t in taps if t[2] > 0], [t for t in taps if t[2] < 0]
    while pos or neg:
        if pos:
            order.append(pos.pop())
        if neg:
            order.append(neg.pop())
    npos = nneg = 0
    for _, _, s in order:
        npos, nneg = npos + (s > 0), nneg + (s < 0)
        assert max(npos, nneg) * 254 < 2048, "f16-exactness bound violated"
    Cp = sum(s for _, _, s in order)
    S4_BIAS = 1536.0 - 63.5 * Cp

    with tile.TileContext(nc) as tc, ExitStack() as ctx:
        consts = ctx.enter_context(tc.tile_pool(name="consts", bufs=1))
        w1 = consts.tile([128, 10], f16)
        w2 = consts.tile([128, 10], f16)
        bias = consts.tile([10, 1], f32)
        nc.sync.dma_start(w1[:], wfc[0:128, :])
        nc.sync.dma_start(w2[:], wfc[128:256, :])
        nc.sync.dma_start(bias[:], fcb[:, :])
        # two persistent padded xh2 buffers; pads = 127.0 forever
        xh2s = [consts.tile([128, TW], f16, name=f"xh2_{i}")
                for i in range(2)]
        for xb in xh2s:
            nc.vector.memset(xb[:], 127.0)

        xpool = ctx.enter_context(tc.tile_pool(name="xp", bufs=2))
        tpool = ctx.enter_context(tc.tile_pool(name="tp", bufs=2))
        vpool = ctx.enter_context(tc.tile_pool(name="vp", bufs=2))
        ypool = ctx.enter_context(tc.tile_pool(name="yp", bufs=2))
        p1pool = ctx.enter_context(tc.tile_pool(name="p1", bufs=2))
        upool = ctx.enter_context(tc.tile_pool(name="up", bufs=2))
        apool = ctx.enter_context(tc.tile_pool(name="ap", bufs=2))
        aTpool = ctx.enter_context(tc.tile_pool(name="aT", bufs=4))
        spool = ctx.enter_context(tc.tile_pool(name="sp", bufs=2))
        po = ctx.enter_context(tc.tile_pool(name="po", bufs=2, space="PSUM"))

        xv_dram = x.rearrange("(g a p) f -> g p a f", p=128, a=G)

        def padded600(t, off):
            # [p, G, 600] view of a padded tile at element offset LEAD+off
            return (t[:, LEAD + off:LEAD + off + G * IS]
                    .rearrange("p (g f) -> p g f", g=G)[:, :, 0:600])

        for g in range(NGROUPS):
            xt = xpool.tile([128, G * 576], f32)
            nc.sync.dma_start(xt[:].rearrange("p (a f) -> p a f", a=G),
                              xv_dram[g])
            t16 = tpool.tile([128, G * 576], f16)
            nc.scalar.activation(t16[:], xt[:], Act.Copy,
                                 bias=1408.0, scale=AF)
            v16 = vpool.tile([128, G * 576], f16)
            nc.scalar.activation(v16[:], t16[:], Act.Relu,
                                 bias=-1409.0, scale=1.0)
            xh2 = xh2s[g % 2]
            xdat = (xh2[:, LEAD:LEAD + G * IS]
                    .rearrange("p (g f) -> p g f", g=G)
                    [:, :, 0:600].rearrange("p g (r c) -> p g r c", r=24)
                    [:, :, :, 0:24])
            nc.scalar.activation(
                xdat, v16[:].rearrange("p (g r c) -> p g r c", g=G, r=24),
                Act.Relu, bias=254.0, scale=-1.0)

            # conv: first op folds two taps, rest accumulate
            yt = ypool.tile([128, TW], f16)
            yv = padded600(yt, 0)
            (dr0, dc0, s0), (dr1, dc1, s1) = order[0], order[1]
            nc.vector.scalar_tensor_tensor(
                yv, padded600(xh2, 25 * dr0 + dc0), s0,
                padded600(xh2, 25 * dr1 + dc1), Alu.mult,
                Alu.add if s1 > 0 else Alu.subtract)
            for dr, dc, s in order[2:]:
                nc.vector.tensor_tensor(
                    yv, yv, padded600(xh2, 25 * dr + dc),
                    Alu.add if s > 0 else Alu.subtract)

            # maxpool 2x2: vertical pairs then horizontal pairs
            yq = (yt[:, LEAD:LEAD + G * IS]
                  .rearrange("p (g f) -> p g f", g=G)
                  [:, :, 0:600].rearrange("p g (rp t c) -> p g rp t c",
                                          t=2, c=25)[:, :, :, :, 0:24])
            p1 = p1pool.tile([128, G * 288], f16)
            p1v = p1[:].rearrange("p (g rp c) -> p g rp c", g=G, rp=12)
            nc.vector.tensor_tensor(p1v, yq[:, :, :, 0, :],
                                    yq[:, :, :, 1, :], Alu.max)
            p1q = p1[:].rearrange("p (g rp c t) -> p g rp c t", g=G, rp=12,
                                  t=2)
            u = upool.tile([128, G * 144], f16)
            uv = u[:].rearrange("p (g rp c) -> p g rp c", g=G, rp=12)
            nc.vector.tensor_tensor(uv, p1q[:, :, :, :, 0],
                                    p1q[:, :, :, :, 1], Alu.max)
            # u = round(128*P) + 1536 ; then clamp to [1536, 1663]
            nc.scalar.activation(u[:], u[:], Act.Copy,
                                 bias=S4_BIAS, scale=0.5)
            act = apool.tile([128, G * 144], f16)
            nc.vector.tensor_scalar(act[:], u[:], 1536.0, 1663.0,
                                    Alu.max, Alu.min)

            # FC: out^T[o, b] = sum_k W[k, o] actT[k, b]
            for h in range(2):
                aT1 = aTpool.tile([128, 512], f16)
                aT2 = aTpool.tile([128, 512], f16)
                for j in range(4):
                    a = h * 4 + j
                    nc.sync.dma_start_transpose(
                        aT1[:, j * 128:(j + 1) * 128],
                        act[:, a * 144:a * 144 + 128])
                    nc.scalar.dma_start_transpose(
                        aT2[:, j * 128:(j + 1) * 128],
                        act[:, a * 144 + 16:a * 144 + 144])
                pOT = po.tile([10, 512], f32)
                nc.tensor.matmul(pOT[:], w1[:], aT1[:], start=True, stop=False)
                nc.tensor.matmul(pOT[:], w2[:], aT2[:], start=False, stop=True)
                soT = spool.tile([10, 512], f32)
                nc.scalar.activation(soT[:], pOT[:], Act.Identity,
                                     bias=bias[:, :], scale=1.0)
                nc.sync.dma_start(
                    out[:, g * 1024 + h * 512:g * 1024 + (h + 1) * 512],
                    soT[:])

    nc.compile()
    return nc


def _prep(conv_w, fc_w):
    # replicate reference weight quantization exactly (all steps exact in f32)
    cw = np.asarray(conv_w, np.float32).reshape(3, 3)
    wq = (np.round(np.clip(cw, -0.5, 0.5) * 2.0) / 2.0).astype(np.float32)
    fw = np.asarray(fc_w, np.float32)
    wfq = (np.round(np.clip(fw, -0.5, 0.5) * 2.0) / 2.0 / 8.0).astype(np.float32)
    # FC sees act values biased by +1536; fold act/128 into W (k/2048, exact
    # fp16) and the bias via fcb = -1536 * sum_k W[k, o].
    # Rows 0:128 = feats 0:128 (W_A); rows 240:256 = feats 128:144 placed at
    # partition 112+ of W_B to match the feats-16:144 transposed tile.
    Wdev = np.zeros((256, 10), np.float32)
    Wfull = np.zeros((144, 10), np.float32)
    for i in range(12):
        for j in range(12):
            k = i * 12 + j
            r = k if k < 128 else k + 112
            col = wfq[:, (i + 1) * 14 + (j + 1)] / 128.0
            Wdev[r, :] = col
            Wfull[k, :] = col
    fcb = (-1536.0 * Wfull.sum(axis=0, dtype=np.float64)).astype(
        np.float32).reshape(10, 1)
    return (tuple(float(v) for v in wq.flatten()), Wdev.astype(np.float16),
            fcb)


def _get_program(wq9):
    nc = _cache.get(wq9)
    if nc is None:
        nc = _build(wq9)
        _cache[wq9] = nc
    return nc


def _make_in_maps(x2d, Wdev, fcb):
    return [{"x": np.ascontiguousarray(x2d[c * NPC:(c + 1) * NPC]),
             "wfc": Wdev, "fcb": fcb} for c in range(NCORES)]


def run(x, conv_w, fc_w, trace=False, **kw):
    from concourse.bass_utils import run_bass_kernel_spmd

    x2d = np.ascontiguousarray(
        np.asarray(x, np.float32).reshape(B, 576))
    wq9, Wdev, fcb = _prep(conv_w, fc_w)
    nc = _get_program(wq9)
    res = run_bass_kernel_spmd(nc, _make_in_maps(x2d, Wdev, fcb),
                               core_ids=list(range(NCORES)),
                               trace=trace, **kw)
    out = np.concatenate([np.asarray(r["out"]).T for r in res.results], axis=0)
    return np.ascontiguousarray(out.astype(np.float32)), res


def kernel(x, conv_w, fc_w):
    out, _ = run(x, conv_w, fc_w, trace=False)
    return out


# revision 21
# speedup vs baseline: 6.4811x; 1.2142x over previous
import sys

if "/opt/trn_rl_repo" not in sys.path:
    sys.path.insert(0, "/opt/trn_rl_repo")

import numpy as np

NCORES = 8
B = 65536
NPC = B // NCORES  # 8192 images per core
G = 8              # image-tiles (of 128) per group
NGROUPS = NPC // (128 * G)
AF = 128.0 / 127.5
IS = 626           # image block: 25 rows x 25 cols + 1 spare (even stride)
LEAD = 32          # leading pad cells (>= 26 so dr=-1,dc=-1 reads stay in-tile)
TW = LEAD + G * IS + 32  # f16 cells per partition for padded tiles

_cache = {}


def _build(wq9):
    """wq9: tuple of 9 floats, quantized conv taps in {0,+-0.5}, row-major.

    Pipeline per group (128 partitions x G images):
      scalar: t = f16(AF*x + 1408)   [= A + 1536; f16 convert rounds to int]
              xh2 = Relu(1663 - t)   [= 127 - min(A,127), upper clamp exact;
                                      A=-128 (0.2% of pixels) maps to 255]
      DVE:    y' = sum_i sigma_i*shift(xh2), sigma=-sign(w), on a 25x25-padded
              layout (pads hold 127 == "A=0").  When conv rows -1/+1 share a
              sign pattern (true for this seed) a shared horizontal term T
              cuts this to 5 tensor ops; else n_taps-1 accumulation ops.
              Then 2x2 maxpool (2 ops) + final clamp; all exact ints in f16.
      scalar: u = f16(0.5*pool + (1536 - 63.5*C))  [= round(128*P) + 1536]
      PE:     FC as K=128 + K=16 matmuls per 512-batch; the +1536 act bias is
              removed via a per-partition bias AP in the PSUM->SBUF copy.
    """
    from contextlib import ExitStack

    import concourse.tile as tile
    from concourse import bacc, mybir

    f32 = mybir.dt.float32
    f16 = mybir.dt.float16
    Alu = mybir.AluOpType
    Act = mybir.ActivationFunctionType

    nc = bacc.Bacc("TRN2", target_bir_lowering=False, debug=False,
                   num_devices=NCORES)

    # non-Copy activation biases need pre-registered const APs
    for cval in (1663.0,):
        ctensor = nc.alloc_sbuf_tensor(f"constb-{cval}", [128, 1], f32)
        nc.gpsimd.memset(ctensor.ap(), cval)
        nc.const_aps.aps[(f32, cval)] = ctensor.ap()
    nc.all_engine_barrier()

    x = nc.dram_tensor("x", [NPC, 576], f32, kind="ExternalInput").ap()
    wfc = nc.dram_tensor("wfc", [128, 180], f16, kind="ExternalInput").ap()
    fcb = nc.dram_tensor("fcb", [10, 1], f32, kind="ExternalInput").ap()
    out = nc.dram_tensor("out", [10, NPC], f32, kind="ExternalOutput").ap()

    # taps: (dr, dc, sigma) with sigma = -sign(w)
    sg = [[(-1.0 if wq9[(dr + 1) * 3 + (dc + 1)] > 0 else
            (1.0 if wq9[(dr + 1) * 3 + (dc + 1)] < 0 else 0.0))
           for dc in (-1, 0, 1)] for dr in (-1, 0, 1)]
    Cp = sum(s for row in sg for s in row)
    S4_BIAS = 1536.0 - 63.5 * Cp
    use_T = (sg[0] == sg[2] and any(s != 0 for s in sg[0]))

    with tile.TileContext(nc) as tc, ExitStack() as ctx:
        consts = ctx.enter_context(tc.tile_pool(name="consts", bufs=1))
        # Per 128-chunk lo/hi zero-padded weight blocks:
        # w1[p, 10*(2c+v)+o] = Wflat[128c+p, o] if row belongs to the lo(v=0)
        # / hi(v=1) image of chunk c, else 0.
        w1 = consts.tile([128, 180], f16)
        bias = consts.tile([10, 1], f32)
        nc.sync.dma_start(w1[:], wfc[:, :])
        nc.sync.dma_start(bias[:], fcb[:, :])
        # persistent padded buffers; pads hold 127.0 forever
        xh2s = [consts.tile([128, TW], f16, name=f"xh2_{i}")
                for i in range(2)]
        Tt = consts.tile([128, TW], f16)
        nc.vector.memset(xh2s[0][:], 127.0)
        nc.vector.memset(xh2s[1][:], 127.0)
        nc.vector.memset(Tt[:], 127.0)

        xpool = ctx.enter_context(tc.tile_pool(name="xp", bufs=2))
        tpool = ctx.enter_context(tc.tile_pool(name="tp", bufs=2))
        ypool = ctx.enter_context(tc.tile_pool(name="yp", bufs=2))
        p1pool = ctx.enter_context(tc.tile_pool(name="p1", bufs=2))
        upool = ctx.enter_context(tc.tile_pool(name="up", bufs=2))
        apool = ctx.enter_context(tc.tile_pool(name="ap", bufs=2))
        aTpool = ctx.enter_context(tc.tile_pool(name="aT", bufs=2))
        spool = ctx.enter_context(tc.tile_pool(name="sp", bufs=2))
        po = ctx.enter_context(tc.tile_pool(name="po", bufs=2, space="PSUM"))

        xv_dram = x.rearrange("(g a p) f -> g p a f", p=128, a=G)

        def pview(t, off, w):
            # [p, G, w] view of a padded tile at element offset LEAD+off
            return (t[:, LEAD + off:LEAD + off + G * IS]
                    .rearrange("p (g f) -> p g f", g=G)[:, :, 0:w])

        for g in range(NGROUPS):
            xt = xpool.tile([128, G * 576], f32)
            nc.sync.dma_start(xt[:].rearrange("p (a f) -> p a f", a=G),
                              xv_dram[g])
            t16 = tpool.tile([128, G * 576], f16)
            nc.scalar.activation(t16[:], xt[:], Act.Copy,
                                 bias=1408.0, scale=AF)
            xh2 = xh2s[g % 2]
            xdat = (pview(xh2, 0, 600)
                    .rearrange("p g (r c) -> p g r c", r=24)[:, :, :, 0:24])
            nc.scalar.activation(
                xdat, t16[:].rearrange("p (g r c) -> p g r c", g=G, r=24),
                Act.Relu, bias=1663.0, scale=-1.0)

            yt = ypool.tile([128, TW], f16)
            yv = pview(yt, 0, 600)
            if use_T:
                # T(rr,c) = sum_dc sg0[dc]*X(rr,c+dc) on the full 625-run
                # (incl. pad row), then y' = T(r-1) + T(r+1) + row-0 taps.
                tv = pview(Tt, 0, 625)
                hh = [(dc, sg[0][dc + 1]) for dc in (-1, 0, 1)
                      if sg[0][dc + 1] != 0]
                (dc0, s0), rest = hh[0], hh[1:]
                if len(hh) >= 2 and s0 > 0:
                    dc1, s1 = rest[0]
                    nc.vector.tensor_tensor(
                        tv, pview(xh2, dc0, 625), pview(xh2, dc1, 625),
                        Alu.add if s1 > 0 else Alu.subtract)
                    rest = rest[1:]
                else:
                    nc.vector.scalar_tensor_tensor(
                        tv, pview(xh2, dc0, 625), s0,
                        pview(xh2, rest[0][0], 625), Alu.mult,
                        Alu.add if rest[0][1] > 0 else Alu.subtract)
                    rest = rest[1:]
                for dc, s in rest:
                    nc.vector.tensor_tensor(
                        tv, tv, pview(xh2, dc, 625),
                        Alu.add if s > 0 else Alu.subtract)
                nc.vector.tensor_tensor(yv, pview(Tt, -25, 600),
                                        pview(Tt, 25, 600), Alu.add)
                mid = [(dc, sg[1][dc + 1]) for dc in (-1, 0, 1)
                       if sg[1][dc + 1] != 0]
                for dc, s in mid:
                    nc.vector.tensor_tensor(
                        yv, yv, pview(xh2, dc, 600),
                        Alu.add if s > 0 else Alu.subtract)
            else:
                order = [(dr, dc, sg[dr + 1][dc + 1])
                         for dr in (-1, 0, 1) for dc in (-1, 0, 1)
                         if sg[dr + 1][dc + 1] != 0]
                order.sort(key=lambda t: -t[2])
                (dr0, dc0, s0), (dr1, dc1, s1) = order[0], order[1]
                if s0 > 0:
                    nc.vector.tensor_tensor(
                        yv, pview(xh2, 25 * dr0 + dc0, 600),
                        pview(xh2, 25 * dr1 + dc1, 600),
                        Alu.add if s1 > 0 else Alu.subtract)
                else:
                    nc.vector.scalar_tensor_tensor(
                        yv, pview(xh2, 25 * dr0 + dc0, 600), s0,
                        pview(xh2, 25 * dr1 + dc1, 600), Alu.mult,
                        Alu.add if s1 > 0 else Alu.subtract)
                for dr, dc, s in order[2:]:
                    nc.vector.tensor_tensor(
                        yv, yv, pview(xh2, 25 * dr + dc, 600),
                        Alu.add if s > 0 else Alu.subtract)

            # maxpool 2x2: vertical pairs then horizontal pairs
            yq = (pview(yt, 0, 600)
                  .rearrange("p g (rp t c) -> p g rp t c",
                             t=2, c=25)[:, :, :, :, 0:24])
            p1 = p1pool.tile([128, G * 288], f16)
            p1v = p1[:].rearrange("p (g rp c) -> p g rp c", g=G, rp=12)
            nc.vector.tensor_tensor(p1v, yq[:, :, :, 0, :],
                                    yq[:, :, :, 1, :], Alu.max)
            p1q = p1[:].rearrange("p (g rp c t) -> p g rp c t", g=G, rp=12,
                                  t=2)
            u = upool.tile([128, G * 144], f16)
            uv = u[:].rearrange("p (g rp c) -> p g rp c", g=G, rp=12)
            nc.vector.tensor_tensor(uv, p1q[:, :, :, :, 0],
                                    p1q[:, :, :, :, 1], Alu.max)
            # u = round(128*P) + 1536 ; then clamp to [1536, 1663]
            nc.scalar.activation(u[:], u[:], Act.Copy,
                                 bias=S4_BIAS, scale=0.5)
            act = apool.tile([128, G * 144], f16)
            nc.vector.tensor_scalar(act[:], u[:], 1536.0, 1663.0,
                                    Alu.max, Alu.min)

            # FC: out^T[o, b] = sum_k W[k, o] actT[k, b].  act is [128, 9*128]
            # flat; 9 non-overlapping transposes, then per image 2 partition-
            # sliced matmuls (its 144 feats span exactly 2 chunks).
            aTf = aTpool.tile([128, G * 144], f16)
            for c in range(9):
                nc.sync.dma_start_transpose(
                    aTf[:, c * 128:(c + 1) * 128],
                    act[:, c * 128:(c + 1) * 128])
            pOT = po.tile([10, G * 128], f32)
            for a in range(G):
                c1, p1 = (144 * a) // 128, (144 * a) % 128
                v1 = 0 if p1 == 0 else 1
                ob = pOT[:, a * 128:(a + 1) * 128]
                nc.tensor.matmul(
                    ob, w1[:, 10 * (2 * c1 + v1):10 * (2 * c1 + v1) + 10],
                    aTf[:, 128 * c1:128 * (c1 + 1)],
                    start=True, stop=False)
                nc.tensor.matmul(
                    ob, w1[:, 10 * (2 * c1 + 2):10 * (2 * c1 + 2) + 10],
                    aTf[:, 128 * (c1 + 1):128 * (c1 + 2)],
                    start=False, stop=True)
            soT = spool.tile([10, G * 128], f32)
            nc.scalar.activation(soT[:], pOT[:], Act.Identity,
                                 bias=bias[:, :], scale=1.0)
            nc.sync.dma_start(out[:, g * 1024:(g + 1) * 1024], soT[:])

    nc.compile()
    return nc


def _prep(conv_w, fc_w):
    # replicate reference weight quantization exactly (all steps exact in f32)
    cw = np.asarray(conv_w, np.float32).reshape(3, 3)
    wq = (np.round(np.clip(cw, -0.5, 0.5) * 2.0) / 2.0).astype(np.float32)
    fw = np.asarray(fc_w, np.float32)
    wfq = (np.round(np.clip(fw, -0.5, 0.5) * 2.0) / 2.0 / 8.0).astype(np.float32)
    # FC sees act values biased by +1536; fold act/128 into W (k/2048, exact
    # fp16) and remove the bias via fcb = -1536 * sum_k W[k, o].
    Wdev = np.zeros((144, 10), np.float32)
    for i in range(12):
        for j in range(12):
            Wdev[i * 12 + j, :] = wfq[:, (i + 1) * 14 + (j + 1)] / 128.0
    fcb = (-1536.0 * Wdev.sum(axis=0, dtype=np.float64)).astype(
        np.float32).reshape(10, 1)
    # flat-col layout: Wflat[144a + k] = Wdev[k]; per chunk c the rows split
    # between two images -> lo/hi zero-padded variants [128, 9*2*10]
    Wflat = np.tile(Wdev, (8, 1))  # [1152, 10]
    Wpack = np.zeros((128, 180), np.float32)
    for c in range(9):
        ac = (128 * c) // 144
        for p in range(128):
            f = 128 * c + p
            v = 0 if f // 144 == ac else 1
            Wpack[p, 10 * (2 * c + v):10 * (2 * c + v) + 10] = Wflat[f]
    return (tuple(float(v) for v in wq.flatten()),
            Wpack.astype(np.float16), fcb)


def _get_program(wq9):
    nc = _cache.get(wq9)
    if nc is None:
        nc = _build(wq9)
        _cache[wq9] = nc
    return nc


def _make_in_maps(x2d, Wdev, fcb):
    return [{"x": np.ascontiguousarray(x2d[c * NPC:(c + 1) * NPC]),
             "wfc": Wdev, "fcb": fcb} for c in range(NCORES)]


def run(x, conv_w, fc_w, trace=False, **kw):
    from concourse.bass_utils import run_bass_kernel_spmd

    x2d = np.ascontiguousarray(
        np.asarray(x, np.float32).reshape(B, 576))
    wq9, Wdev, fcb = _prep(conv_w, fc_w)
    nc = _get_program(wq9)
    res = run_bass_kernel_spmd(nc, _make_in_maps(x2d, Wdev, fcb),
                               core_ids=list(range(NCORES)),
                               trace=trace, **kw)
    out = np.concatenate([np.asarray(r["out"]).T for r in res.results], axis=0)
    return np.ascontiguousarray(out.astype(np.float32)), res


def kernel(x, conv_w, fc_w):
    out, _ = run(x, conv_w, fc_w, trace=False)
    return out


# revision 24
# speedup vs baseline: 6.5341x; 1.0082x over previous
import sys

if "/opt/trn_rl_repo" not in sys.path:
    sys.path.insert(0, "/opt/trn_rl_repo")

import numpy as np

NCORES = 8
B = 65536
NPC = B // NCORES  # 8192 images per core
G = 8              # image-tiles (of 128) per group
NGROUPS = NPC // (128 * G)
AF = 128.0 / 127.5
IS = 626           # image block: 25 rows x 25 cols + 1 spare (even stride)
LEAD = 32          # leading pad cells (>= 26 so dr=-1,dc=-1 reads stay in-tile)
TW = LEAD + G * IS + 32  # f16 cells per partition for padded tiles

_cache = {}


def _build(wq9):
    """wq9: tuple of 9 floats, quantized conv taps in {0,+-0.5}, row-major.

    Pipeline per group (128 partitions x G images):
      scalar: t = f16(AF*x + 1408)   [= A + 1536; f16 convert rounds to int]
              xh2 = Relu(1663 - t)   [= 127 - min(A,127), upper clamp exact;
                                      A=-128 (0.2% of pixels) maps to 255]
      DVE:    y' = sum_i sigma_i*shift(xh2), sigma=-sign(w), on a 25x25-padded
              layout (pads hold 127 == "A=0").  When conv rows -1/+1 share a
              sign pattern (true for this seed) a shared horizontal term T
              cuts this to 5 tensor ops; else n_taps-1 accumulation ops.
              Then 2x2 maxpool (2 ops) + final clamp; all exact ints in f16.
      scalar: u = f16(0.5*pool + (1536 - 63.5*C))  [= round(128*P) + 1536]
      PE:     FC as K=128 + K=16 matmuls per 512-batch; the +1536 act bias is
              removed via a per-partition bias AP in the PSUM->SBUF copy.
    """
    from contextlib import ExitStack

    import concourse.tile as tile
    from concourse import bacc, mybir

    f32 = mybir.dt.float32
    f16 = mybir.dt.float16
    Alu = mybir.AluOpType
    Act = mybir.ActivationFunctionType

    nc = bacc.Bacc("TRN2", target_bir_lowering=False, debug=False,
                   num_devices=NCORES)

    # non-Copy activation biases need pre-registered const APs
    for cval in (1663.0,):
        ctensor = nc.alloc_sbuf_tensor(f"constb-{cval}", [128, 1], f32)
        nc.gpsimd.memset(ctensor.ap(), cval)
        nc.const_aps.aps[(f32, cval)] = ctensor.ap()
    nc.all_engine_barrier()

    x = nc.dram_tensor("x", [NPC, 576], f32, kind="ExternalInput").ap()
    wfc = nc.dram_tensor("wfc", [128, 180], f16, kind="ExternalInput").ap()
    fcb = nc.dram_tensor("fcb", [10, 1], f32, kind="ExternalInput").ap()
    out = nc.dram_tensor("out", [10, NPC], f32, kind="ExternalOutput").ap()

    # taps: (dr, dc, sigma) with sigma = -sign(w)
    sg = [[(-1.0 if wq9[(dr + 1) * 3 + (dc + 1)] > 0 else
            (1.0 if wq9[(dr + 1) * 3 + (dc + 1)] < 0 else 0.0))
           for dc in (-1, 0, 1)] for dr in (-1, 0, 1)]
    Cp = sum(s for row in sg for s in row)
    S4_BIAS = 1536.0 - 63.5 * Cp
    use_T = (sg[0] == sg[2] and any(s != 0 for s in sg[0]))

    with tile.TileContext(nc) as tc, ExitStack() as ctx:
        consts = ctx.enter_context(tc.tile_pool(name="consts", bufs=1))
        # Per 128-chunk lo/hi zero-padded weight blocks:
        # w1[p, 10*(2c+v)+o] = Wflat[128c+p, o] if row belongs to the lo(v=0)
        # / hi(v=1) image of chunk c, else 0.
        w1 = consts.tile([128, 180], f16)
        bias = consts.tile([10, 1], f32)
        nc.sync.dma_start(w1[:], wfc[:, :])
        nc.sync.dma_start(bias[:], fcb[:, :])
        # persistent padded buffers; pads hold 127.0 forever
        xh2s = [consts.tile([128, TW], f16, name=f"xh2_{i}")
                for i in range(2)]
        Tt = consts.tile([128, TW], f16)
        nc.vector.memset(xh2s[0][:], 127.0)
        nc.vector.memset(xh2s[1][:], 127.0)
        nc.vector.memset(Tt[:], 127.0)

        xpool = ctx.enter_context(tc.tile_pool(name="xp", bufs=2))
        tpool = ctx.enter_context(tc.tile_pool(name="tp", bufs=2))
        ypool = ctx.enter_context(tc.tile_pool(name="yp", bufs=2))
        p1pool = ctx.enter_context(tc.tile_pool(name="p1", bufs=2))
        upool = ctx.enter_context(tc.tile_pool(name="up", bufs=2))
        apool = ctx.enter_context(tc.tile_pool(name="ap", bufs=2))
        aTpool = ctx.enter_context(tc.tile_pool(name="aT", bufs=2))
        spool = ctx.enter_context(tc.tile_pool(name="sp", bufs=2))
        po = ctx.enter_context(tc.tile_pool(name="po", bufs=2, space="PSUM"))

        xv_dram = x.rearrange("(g a p) f -> g p a f", p=128, a=G)

        def pview(t, off, w):
            # [p, G, w] view of a padded tile at element offset LEAD+off
            return (t[:, LEAD + off:LEAD + off + G * IS]
                    .rearrange("p (g f) -> p g f", g=G)[:, :, 0:w])

        acts = {}

        def quant(g):
            xt = xpool.tile([128, G * 576], f32, name="xt")
            nc.sync.dma_start(xt[:].rearrange("p (a f) -> p a f", a=G),
                              xv_dram[g])
            t16 = tpool.tile([128, G * 576], f16, name="t16")
            nc.scalar.activation(t16[:], xt[:], Act.Copy,
                                 bias=1408.0, scale=AF)
            xh2 = xh2s[g % 2]
            xdat = (pview(xh2, 0, 600)
                    .rearrange("p g (r c) -> p g r c", r=24)[:, :, :, 0:24])
            nc.scalar.activation(
                xdat, t16[:].rearrange("p (g r c) -> p g r c", g=G, r=24),
                Act.Relu, bias=1663.0, scale=-1.0)

        def core(g):
            xh2 = xh2s[g % 2]
            yt = ypool.tile([128, TW], f16, name="yt")
            yv = pview(yt, 0, 600)
            if use_T:
                # T(rr,c) = sum_dc sg0[dc]*X(rr,c+dc) on the full 625-run
                # (incl. pad row), then y' = T(r-1) + T(r+1) + row-0 taps.
                tv = pview(Tt, 0, 625)
                hh = [(dc, sg[0][dc + 1]) for dc in (-1, 0, 1)
                      if sg[0][dc + 1] != 0]
                (dc0, s0), rest = hh[0], hh[1:]
                if len(hh) >= 2 and s0 > 0:
                    dc1, s1 = rest[0]
                    nc.vector.tensor_tensor(
                        tv, pview(xh2, dc0, 625), pview(xh2, dc1, 625),
                        Alu.add if s1 > 0 else Alu.subtract)
                    rest = rest[1:]
                else:
                    nc.vector.scalar_tensor_tensor(
                        tv, pview(xh2, dc0, 625), s0,
                        pview(xh2, rest[0][0], 625), Alu.mult,
                        Alu.add if rest[0][1] > 0 else Alu.subtract)
                    rest = rest[1:]
                for dc, s in rest:
                    nc.vector.tensor_tensor(
                        tv, tv, pview(xh2, dc, 625),
                        Alu.add if s > 0 else Alu.subtract)
                nc.vector.tensor_tensor(yv, pview(Tt, -25, 600),
                                        pview(Tt, 25, 600), Alu.add)
                mid = [(dc, sg[1][dc + 1]) for dc in (-1, 0, 1)
                       if sg[1][dc + 1] != 0]
                for dc, s in mid:
                    nc.vector.tensor_tensor(
                        yv, yv, pview(xh2, dc, 600),
                        Alu.add if s > 0 else Alu.subtract)
            else:
                order = [(dr, dc, sg[dr + 1][dc + 1])
                         for dr in (-1, 0, 1) for dc in (-1, 0, 1)
                         if sg[dr + 1][dc + 1] != 0]
                order.sort(key=lambda t: -t[2])
                (dr0, dc0, s0), (dr1, dc1, s1) = order[0], order[1]
                if s0 > 0:
                    nc.vector.tensor_tensor(
                        yv, pview(xh2, 25 * dr0 + dc0, 600),
                        pview(xh2, 25 * dr1 + dc1, 600),
                        Alu.add if s1 > 0 else Alu.subtract)
                else:
                    nc.vector.scalar_tensor_tensor(
                        yv, pview(xh2, 25 * dr0 + dc0, 600), s0,
                        pview(xh2, 25 * dr1 + dc1, 600), Alu.mult,
                        Alu.add if s1 > 0 else Alu.subtract)
                for dr, dc, s in order[2:]:
                    nc.vector.tensor_tensor(
                        yv, yv, pview(xh2, 25 * dr + dc, 600),
                        Alu.add if s > 0 else Alu.subtract)

            # maxpool 2x2: vertical pairs then horizontal pairs
            yq = (pview(yt, 0, 600)
                  .rearrange("p g (rp t c) -> p g rp t c",
                             t=2, c=25)[:, :, :, :, 0:24])
            p1 = p1pool.tile([128, G * 288], f16, name="p1t")
            p1v = p1[:].rearrange("p (g rp c) -> p g rp c", g=G, rp=12)
            nc.vector.tensor_tensor(p1v, yq[:, :, :, 0, :],
                                    yq[:, :, :, 1, :], Alu.max)
            p1q = p1[:].rearrange("p (g rp c t) -> p g rp c t", g=G, rp=12,
                                  t=2)
            u = upool.tile([128, G * 144], f16, name="ut")
            uv = u[:].rearrange("p (g rp c) -> p g rp c", g=G, rp=12)
            nc.vector.tensor_tensor(uv, p1q[:, :, :, :, 0],
                                    p1q[:, :, :, :, 1], Alu.max)
            # u = round(128*P) + 1536 ; then clamp to [1536, 1663]
            nc.scalar.activation(u[:], u[:], Act.Copy,
                                 bias=S4_BIAS, scale=0.5)
            act = apool.tile([128, G * 144], f16, name="actt")
            nc.vector.tensor_scalar(act[:], u[:], 1536.0, 1663.0,
                                    Alu.max, Alu.min)
            acts[g] = act

        def fc(g):
            # FC: out^T[o, b] = sum_k W[k, o] actT[k, b].  act is [128, 9*128]
            # flat; 9 non-overlapping transposes, then per image 2 matmuls
            # with lo/hi zero-padded weights (its 144 feats span 2 chunks).
            act = acts.pop(g)
            aTf = aTpool.tile([128, G * 144], f16, name="aTf")
            for c in range(9):
                nc.sync.dma_start_transpose(
                    aTf[:, c * 128:(c + 1) * 128],
                    act[:, c * 128:(c + 1) * 128])
            pOT = po.tile([10, G * 128], f32, name="pOTt")
            for a in range(G):
                c1, p1 = (144 * a) // 128, (144 * a) % 128
                v1 = 0 if p1 == 0 else 1
                ob = pOT[:, a * 128:(a + 1) * 128]
                nc.tensor.matmul(
                    ob, w1[:, 10 * (2 * c1 + v1):10 * (2 * c1 + v1) + 10],
                    aTf[:, 128 * c1:128 * (c1 + 1)],
                    start=True, stop=False)
                nc.tensor.matmul(
                    ob, w1[:, 10 * (2 * c1 + 2):10 * (2 * c1 + 2) + 10],
                    aTf[:, 128 * (c1 + 1):128 * (c1 + 2)],
                    start=False, stop=True)
            soT = spool.tile([10, G * 128], f32, name="soTt")
            nc.scalar.activation(soT[:], pOT[:], Act.Identity,
                                 bias=bias[:, :], scale=1.0)
            nc.sync.dma_start(out[:, g * 1024:(g + 1) * 1024], soT[:])

        # 3-stage software pipeline: quant(g) | core(g-1) | fc(g-2), so no
        # engine's in-order queue blocks the next group's prerequisites.
        for g in range(NGROUPS + 2):
            if g < NGROUPS:
                quant(g)
            if 1 <= g <= NGROUPS:
                core(g - 1)
            if g >= 2:
                fc(g - 2)

    nc.compile()
    return nc


def _prep(conv_w, fc_w):
    # replicate reference weight quantization exactly (all steps exact in f32)
    cw = np.asarray(conv_w, np.float32).reshape(3, 3)
    wq = (np.round(np.clip(cw, -0.5, 0.5) * 2.0) / 2.0).astype(np.float32)
    fw = np.asarray(fc_w, np.float32)
    wfq = (np.round(np.clip(fw, -0.5, 0.5) * 2.0) / 2.0 / 8.0).astype(np.float32)
    # FC sees act values biased by +1536; fold act/128 into W (k/2048, exact
    # fp16) and remove the bias via fcb = -1536 * sum_k W[k, o].
    Wdev = np.zeros((144, 10), np.float32)
    for i in range(12):
        for j in range(12):
            Wdev[i * 12 + j, :] = wfq[:, (i + 1) * 14 + (j + 1)] / 128.0
    fcb = (-1536.0 * Wdev.sum(axis=0, dtype=np.float64)).astype(
        np.float32).reshape(10, 1)
    # flat-col layout: Wflat[144a + k] = Wdev[k]; per chunk c the rows split
    # between two images -> lo/hi zero-padded variants [128, 9*2*10]
    Wflat = np.tile(Wdev, (8, 1))  # [1152, 10]
    Wpack = np.zeros((128, 180), np.float32)
    for c in range(9):
        ac = (128 * c) // 144
        for p in range(128):
            f = 128 * c + p
            v = 0 if f // 144 == ac else 1
            Wpack[p, 10 * (2 * c + v):10 * (2 * c + v) + 10] = Wflat[f]
    return (tuple(float(v) for v in wq.flatten()),
            Wpack.astype(np.float16), fcb)


def _get_program(wq9):
    nc = _cache.get(wq9)
    if nc is None:
        nc = _build(wq9)
        _cache[wq9] = nc
    return nc


def _make_in_maps(x2d, Wdev, fcb):
    return [{"x": np.ascontiguousarray(x2d[c * NPC:(c + 1) * NPC]),
             "wfc": Wdev, "fcb": fcb} for c in range(NCORES)]


def run(x, conv_w, fc_w, trace=False, **kw):
    from concourse.bass_utils import run_bass_kernel_spmd

    x2d = np.ascontiguousarray(
        np.asarray(x, np.float32).reshape(B, 576))
    wq9, Wdev, fcb = _prep(conv_w, fc_w)
    nc = _get_program(wq9)
    res = run_bass_kernel_spmd(nc, _make_in_maps(x2d, Wdev, fcb),
                               core_ids=list(range(NCORES)),
                               trace=trace, **kw)
    out = np.concatenate([np.asarray(r["out"]).T for r in res.results], axis=0)
    return np.ascontiguousarray(out.astype(np.float32)), res


def kernel(x, conv_w, fc_w):
    out, _ = run(x, conv_w, fc_w, trace=False)
    return out


# revision 29
# speedup vs baseline: 7.1084x; 1.0879x over previous
import sys

if "/opt/trn_rl_repo" not in sys.path:
    sys.path.insert(0, "/opt/trn_rl_repo")

import numpy as np

NCORES = 8
B = 65536
NPC = B // NCORES  # 8192 images per core
G = 8              # image-tiles (of 128) per group
NGROUPS = NPC // (128 * G)
AF = 128.0 / 127.5
IS = 626           # image block: 25 rows x 25 cols + 1 spare (even stride)
LEAD = 32          # leading pad cells (>= 26 so dr=-1,dc=-1 reads stay in-tile)
TW = LEAD + G * IS + 32  # f16 cells per partition for padded tiles

_cache = {}


def _build(wq9):
    """wq9: tuple of 9 floats, quantized conv taps in {0,+-0.5}, row-major.

    Pipeline per group (128 partitions x G images):
      scalar: t = f16(AF*x + 1408)   [= A + 1536; f16 convert rounds to int]
              xh2 = Relu(1663 - t)   [= 127 - min(A,127), upper clamp exact;
                                      A=-128 (0.2% of pixels) maps to 255]
      DVE:    y' = sum_i sigma_i*shift(xh2), sigma=-sign(w), on a 25x25-padded
              layout (pads hold 127 == "A=0").  When conv rows -1/+1 share a
              sign pattern (true for this seed) a shared horizontal term T
              cuts this to 5 tensor ops; else n_taps-1 accumulation ops.
              Then 2x2 maxpool (2 ops) + final clamp; all exact ints in f16.
      scalar: u = f16(0.5*pool + (1536 - 63.5*C))  [= round(128*P) + 1536]
      PE:     FC as K=128 + K=16 matmuls per 512-batch; the +1536 act bias is
              removed via a per-partition bias AP in the PSUM->SBUF copy.
    """
    from contextlib import ExitStack

    import concourse.tile as tile
    from concourse import bacc, mybir

    f32 = mybir.dt.float32
    f16 = mybir.dt.float16
    Alu = mybir.AluOpType
    Act = mybir.ActivationFunctionType

    nc = bacc.Bacc("TRN2", target_bir_lowering=False, debug=False,
                   num_devices=NCORES)

    # non-Copy activation biases need pre-registered const APs
    for cval in (1663.0,):
        ctensor = nc.alloc_sbuf_tensor(f"constb-{cval}", [128, 1], f32)
        nc.gpsimd.memset(ctensor.ap(), cval)
        nc.const_aps.aps[(f32, cval)] = ctensor.ap()
    nc.all_engine_barrier()

    x = nc.dram_tensor("x", [NPC, 576], f32, kind="ExternalInput").ap()
    wfc = nc.dram_tensor("wfc", [128, 180], f16, kind="ExternalInput").ap()
    fcb = nc.dram_tensor("fcb", [10, 1], f32, kind="ExternalInput").ap()
    out = nc.dram_tensor("out", [10, NPC], f32, kind="ExternalOutput").ap()

    # taps: (dr, dc, sigma) with sigma = -sign(w)
    sg = [[(-1.0 if wq9[(dr + 1) * 3 + (dc + 1)] > 0 else
            (1.0 if wq9[(dr + 1) * 3 + (dc + 1)] < 0 else 0.0))
           for dc in (-1, 0, 1)] for dr in (-1, 0, 1)]
    Cp = sum(s for row in sg for s in row)
    S4_BIAS = 1536.0 - 63.5 * Cp
    use_T = (sg[0] == sg[2] and any(s != 0 for s in sg[0]))

    with tile.TileContext(nc) as tc, ExitStack() as ctx:
        consts = ctx.enter_context(tc.tile_pool(name="consts", bufs=1))
        # Per 128-chunk lo/hi zero-padded weight blocks:
        # w1[p, 10*(2c+v)+o] = Wflat[128c+p, o] if row belongs to the lo(v=0)
        # / hi(v=1) image of chunk c, else 0.
        w1 = consts.tile([128, 180], f16)
        bias = consts.tile([10, 1], f32)
        nc.sync.dma_start(w1[:], wfc[:, :])
        nc.sync.dma_start(bias[:], fcb[:, :])
        # persistent padded buffers; pads hold 127.0 forever
        xh2s = [consts.tile([128, TW], f16, name=f"xh2_{i}")
                for i in range(2)]
        Tt = consts.tile([128, TW], f16)
        nc.vector.memset(xh2s[0][:], 127.0)
        nc.vector.memset(xh2s[1][:], 127.0)
        nc.vector.memset(Tt[:], 127.0)

        xpool = ctx.enter_context(tc.tile_pool(name="xp", bufs=2))
        tpool = ctx.enter_context(tc.tile_pool(name="tp", bufs=2))
        ypool = ctx.enter_context(tc.tile_pool(name="yp", bufs=2))
        p1pool = ctx.enter_context(tc.tile_pool(name="p1", bufs=2))
        upool = ctx.enter_context(tc.tile_pool(name="up", bufs=2))
        apool = ctx.enter_context(tc.tile_pool(name="ap", bufs=2))
        aTpool = ctx.enter_context(tc.tile_pool(name="aT", bufs=2))
        spool = ctx.enter_context(tc.tile_pool(name="sp", bufs=2))
        po = ctx.enter_context(tc.tile_pool(name="po", bufs=2, space="PSUM"))

        xv_dram = x.rearrange("(g a p) f -> g p a f", p=128, a=G)

        def pview(t, off, w):
            # [p, G, w] view of a padded tile at element offset LEAD+off
            return (t[:, LEAD + off:LEAD + off + G * IS]
                    .rearrange("p (g f) -> p g f", g=G)[:, :, 0:w])

        acts = {}

        def quant(g, halves=1):
            xt = xpool.tile([128, G * 576], f32, name="xt")
            t16 = tpool.tile([128, G * 576], f16, name="t16")
            xh2 = xh2s[g % 2]
            xdat = (pview(xh2, 0, 600)
                    .rearrange("p g (r c) -> p g r c", r=24)[:, :, :, 0:24])
            xtv = xt[:].rearrange("p (a f) -> p a f", a=G)
            t16v = t16[:].rearrange("p (a f) -> p a f", a=G)
            t16q = t16[:].rearrange("p (g r c) -> p g r c", g=G, r=24)
            h = G // halves
            for i in range(halves):
                s = slice(i * h, (i + 1) * h)
                nc.sync.dma_start(xtv[:, s], xv_dram[g][:, s])
                nc.scalar.activation(t16v[:, s], xtv[:, s], Act.Copy,
                                     bias=1408.0, scale=AF)
                nc.scalar.activation(xdat[:, s], t16q[:, s],
                                     Act.Relu, bias=1663.0, scale=-1.0)

        def core(g):
            xh2 = xh2s[g % 2]
            yt = ypool.tile([128, TW], f16, name="yt")
            yv = pview(yt, 0, 600)
            if use_T:
                # T(rr,c) = sum_dc sg0[dc]*X(rr,c+dc) on the full 625-run
                # (incl. pad row), then y' = T(r-1) + T(r+1) + row-0 taps.
                tv = pview(Tt, 0, 625)
                hh = [(dc, sg[0][dc + 1]) for dc in (-1, 0, 1)
                      if sg[0][dc + 1] != 0]
                (dc0, s0), rest = hh[0], hh[1:]
                if len(hh) >= 2 and s0 > 0:
                    dc1, s1 = rest[0]
                    nc.vector.tensor_tensor(
                        tv, pview(xh2, dc0, 625), pview(xh2, dc1, 625),
                        Alu.add if s1 > 0 else Alu.subtract)
                    rest = rest[1:]
                else:
                    nc.vector.scalar_tensor_tensor(
                        tv, pview(xh2, dc0, 625), s0,
                        pview(xh2, rest[0][0], 625), Alu.mult,
                        Alu.add if rest[0][1] > 0 else Alu.subtract)
                    rest = rest[1:]
                for dc, s in rest:
                    nc.vector.tensor_tensor(
                        tv, tv, pview(xh2, dc, 625),
                        Alu.add if s > 0 else Alu.subtract)
                nc.vector.tensor_tensor(yv, pview(Tt, -25, 600),
                                        pview(Tt, 25, 600), Alu.add)
                mid = [(dc, sg[1][dc + 1]) for dc in (-1, 0, 1)
                       if sg[1][dc + 1] != 0]
                for dc, s in mid:
                    nc.vector.tensor_tensor(
                        yv, yv, pview(xh2, dc, 600),
                        Alu.add if s > 0 else Alu.subtract)
            else:
                order = [(dr, dc, sg[dr + 1][dc + 1])
                         for dr in (-1, 0, 1) for dc in (-1, 0, 1)
                         if sg[dr + 1][dc + 1] != 0]
                order.sort(key=lambda t: -t[2])
                (dr0, dc0, s0), (dr1, dc1, s1) = order[0], order[1]
                if s0 > 0:
                    nc.vector.tensor_tensor(
                        yv, pview(xh2, 25 * dr0 + dc0, 600),
                        pview(xh2, 25 * dr1 + dc1, 600),
                        Alu.add if s1 > 0 else Alu.subtract)
                else:
                    nc.vector.scalar_tensor_tensor(
                        yv, pview(xh2, 25 * dr0 + dc0, 600), s0,
                        pview(xh2, 25 * dr1 + dc1, 600), Alu.mult,
                        Alu.add if s1 > 0 else Alu.subtract)
                for dr, dc, s in order[2:]:
                    nc.vector.tensor_tensor(
                        yv, yv, pview(xh2, 25 * dr + dc, 600),
                        Alu.add if s > 0 else Alu.subtract)

            # maxpool 2x2: vertical pairs then horizontal pairs
            yq = (pview(yt, 0, 600)
                  .rearrange("p g (rp t c) -> p g rp t c",
                             t=2, c=25)[:, :, :, :, 0:24])
            p1 = p1pool.tile([128, G * 288], f16, name="p1t")
            p1v = p1[:].rearrange("p (g rp c) -> p g rp c", g=G, rp=12)
            nc.vector.tensor_tensor(p1v, yq[:, :, :, 0, :],
                                    yq[:, :, :, 1, :], Alu.max)
            p1q = p1[:].rearrange("p (g rp c t) -> p g rp c t", g=G, rp=12,
                                  t=2)
            u = upool.tile([128, G * 144], f16, name="ut")
            uv = u[:].rearrange("p (g rp c) -> p g rp c", g=G, rp=12)
            nc.vector.tensor_tensor(uv, p1q[:, :, :, :, 0],
                                    p1q[:, :, :, :, 1], Alu.max)
            # u = round(128*P) + 1536 (f16 write rounds); clamp [1536, 1663]
            nc.vector.tensor_scalar(u[:], u[:], 0.5, S4_BIAS,
                                    Alu.mult, Alu.add)
            act = apool.tile([128, G * 144], f16, name="actt")
            nc.vector.tensor_scalar(act[:], u[:], 1536.0, 1663.0,
                                    Alu.max, Alu.min)
            # issue transposes now so fc(g)'s matmuls find them done.
            # act is [128, 9*128] flat; 9 non-overlapping transposes; per
            # image 2 matmuls w/ lo/hi zero-padded weights (144 feats span
            # 2 chunks).
            aTf = aTpool.tile([128, G * 144], f16, name="aTf")
            for c in range(9):
                nc.sync.dma_start_transpose(
                    aTf[:, c * 128:(c + 1) * 128],
                    act[:, c * 128:(c + 1) * 128])
            acts[g] = aTf

        def fc(g):
            aTf = acts.pop(g)
            pOT = po.tile([10, G * 128], f32, name="pOTt")
            for a in range(G):
                c1, p1 = (144 * a) // 128, (144 * a) % 128
                v1 = 0 if p1 == 0 else 1
                ob = pOT[:, a * 128:(a + 1) * 128]
                nc.tensor.matmul(
                    ob, w1[:, 10 * (2 * c1 + v1):10 * (2 * c1 + v1) + 10],
                    aTf[:, 128 * c1:128 * (c1 + 1)],
                    start=True, stop=False)
                nc.tensor.matmul(
                    ob, w1[:, 10 * (2 * c1 + 2):10 * (2 * c1 + 2) + 10],
                    aTf[:, 128 * (c1 + 1):128 * (c1 + 2)],
                    start=False, stop=True)
            soT = spool.tile([10, G * 128], f32, name="soTt")
            nc.scalar.activation(soT[:], pOT[:], Act.Identity,
                                 bias=bias[:, :], scale=1.0)
            nc.sync.dma_start(out[:, g * 1024:(g + 1) * 1024], soT[:])

        # 3-stage software pipeline: quant(g) | core(g-1) | fc(g-2), so no
        # engine's in-order queue blocks the next group's prerequisites.
        for g in range(NGROUPS + 2):
            if g < NGROUPS:
                quant(g, halves=2 if g == 0 else 1)
            if 1 <= g <= NGROUPS:
                core(g - 1)
            if g >= 2:
                fc(g - 2)

    nc.compile()
    return nc


def _prep(conv_w, fc_w):
    # replicate reference weight quantization exactly (all steps exact in f32)
    cw = np.asarray(conv_w, np.float32).reshape(3, 3)
    wq = (np.round(np.clip(cw, -0.5, 0.5) * 2.0) / 2.0).astype(np.float32)
    fw = np.asarray(fc_w, np.float32)
    wfq = (np.round(np.clip(fw, -0.5, 0.5) * 2.0) / 2.0 / 8.0).astype(np.float32)
    # FC sees act values biased by +1536; fold act/128 into W (k/2048, exact
    # fp16) and remove the bias via fcb = -1536 * sum_k W[k, o].
    Wdev = np.zeros((144, 10), np.float32)
    for i in range(12):
        for j in range(12):
            Wdev[i * 12 + j, :] = wfq[:, (i + 1) * 14 + (j + 1)] / 128.0
    fcb = (-1536.0 * Wdev.sum(axis=0, dtype=np.float64)).astype(
        np.float32).reshape(10, 1)
    # flat-col layout: Wflat[144a + k] = Wdev[k]; per chunk c the rows split
    # between two images -> lo/hi zero-padded variants [128, 9*2*10]
    Wflat = np.tile(Wdev, (8, 1))  # [1152, 10]
    Wpack = np.zeros((128, 180), np.float32)
    for c in range(9):
        ac = (128 * c) // 144
        for p in range(128):
            f = 128 * c + p
            v = 0 if f // 144 == ac else 1
            Wpack[p, 10 * (2 * c + v):10 * (2 * c + v) + 10] = Wflat[f]
    return (tuple(float(v) for v in wq.flatten()),
            Wpack.astype(np.float16), fcb)


def _get_program(wq9):
    nc = _cache.get(wq9)
    if nc is None:
        nc = _build(wq9)
        _cache[wq9] = nc
    return nc


def _make_in_maps(x2d, Wdev, fcb):
    return [{"x": np.ascontiguousarray(x2d[c * NPC:(c + 1) * NPC]),
             "wfc": Wdev, "fcb": fcb} for c in range(NCORES)]


def run(x, conv_w, fc_w, trace=False, **kw):
    from concourse.bass_utils import run_bass_kernel_spmd

    x2d = np.ascontiguousarray(
        np.asarray(x, np.float32).reshape(B, 576))
    wq9, Wdev, fcb = _prep(conv_w, fc_w)
    nc = _get_program(wq9)
    res = run_bass_kernel_spmd(nc, _make_in_maps(x2d, Wdev, fcb),
                               core_ids=list(range(NCORES)),
                               trace=trace, **kw)
    out = np.concatenate([np.asarray(r["out"]).T for r in res.results], axis=0)
    return np.ascontiguousarray(out.astype(np.float32)), res


def kernel(x, conv_w, fc_w):
    out, _ = run(x, conv_w, fc_w, trace=False)
    return out


# revision 35
# speedup vs baseline: 8.1851x; 1.1515x over previous
import sys

if "/opt/trn_rl_repo" not in sys.path:
    sys.path.insert(0, "/opt/trn_rl_repo")

import numpy as np

NCORES = 8
B = 65536
NPC = B // NCORES  # 8192 images per core
G = 8              # image-tiles (of 128) per group
NGROUPS = NPC // (128 * G)
AF = 128.0 / 127.5
IS = 626           # image block: 25 rows x 25 cols + 1 spare (even stride)
LEAD = 32          # leading pad cells (>= 26 so dr=-1,dc=-1 reads stay in-tile)
TW = LEAD + G * IS + 32  # f16 cells per partition for padded tiles

_cache = {}


def _build(wq9):
    """wq9: tuple of 9 floats, quantized conv taps in {0,+-0.5}, row-major.

    Pipeline per group (128 partitions x G images):
      scalar: t = f16(AF*x + 1408)   [= A + 1536; f16 convert rounds to int]
              xh2 = Relu(1663 - t)   [= 127 - min(A,127), upper clamp exact;
                                      A=-128 (0.2% of pixels) maps to 255]
      DVE:    y' = sum_i sigma_i*shift(xh2), sigma=-sign(w), on a 25x25-padded
              layout (pads hold 127 == "A=0").  When conv rows -1/+1 share a
              sign pattern (true for this seed) a shared horizontal term T
              cuts this to 5 tensor ops; else n_taps-1 accumulation ops.
              Then 2x2 maxpool (2 ops) + final clamp; all exact ints in f16.
      scalar: u = f16(0.5*pool + (1536 - 63.5*C))  [= round(128*P) + 1536]
      PE:     FC as K=128 + K=16 matmuls per 512-batch; the +1536 act bias is
              removed via a per-partition bias AP in the PSUM->SBUF copy.
    """
    from contextlib import ExitStack

    import concourse.tile as tile
    from concourse import bacc, mybir

    f32 = mybir.dt.float32
    f16 = mybir.dt.float16
    Alu = mybir.AluOpType
    Act = mybir.ActivationFunctionType

    nc = bacc.Bacc("TRN2", target_bir_lowering=False, debug=False,
                   num_devices=NCORES)

    # non-Copy activation biases need pre-registered const APs
    for cval in (1663.0,):
        ctensor = nc.alloc_sbuf_tensor(f"constb-{cval}", [128, 1], f32)
        nc.gpsimd.memset(ctensor.ap(), cval)
        nc.const_aps.aps[(f32, cval)] = ctensor.ap()
    nc.all_engine_barrier()

    x = nc.dram_tensor("x", [NPC, 576], f32, kind="ExternalInput").ap()
    wfc = nc.dram_tensor("wfc", [128, 180], f16, kind="ExternalInput").ap()
    fcb = nc.dram_tensor("fcb", [10, 1], f32, kind="ExternalInput").ap()
    idn = nc.dram_tensor("idn", [128, 128], f16, kind="ExternalInput").ap()
    out = nc.dram_tensor("out", [10, NPC], f32, kind="ExternalOutput").ap()

    # taps: (dr, dc, sigma) with sigma = -sign(w)
    sg = [[(-1.0 if wq9[(dr + 1) * 3 + (dc + 1)] > 0 else
            (1.0 if wq9[(dr + 1) * 3 + (dc + 1)] < 0 else 0.0))
           for dc in (-1, 0, 1)] for dr in (-1, 0, 1)]
    Cp = sum(s for row in sg for s in row)
    S4_BIAS = 1536.0 - 63.5 * Cp
    use_T = (sg[0] == sg[2] and any(s != 0 for s in sg[0]))

    with tile.TileContext(nc) as tc, ExitStack() as ctx:
        consts = ctx.enter_context(tc.tile_pool(name="consts", bufs=1))
        # Per 128-chunk lo/hi zero-padded weight blocks:
        # w1[p, 10*(2c+v)+o] = Wflat[128c+p, o] if row belongs to the lo(v=0)
        # / hi(v=1) image of chunk c, else 0.
        w1 = consts.tile([128, 180], f16)
        bias = consts.tile([10, 1], f32)
        ident = consts.tile([128, 128], f16)
        nc.sync.dma_start(w1[:], wfc[:, :])
        nc.sync.dma_start(bias[:], fcb[:, :])
        nc.sync.dma_start(ident[:], idn[:, :])
        # persistent padded buffers; pads hold 127.0 forever
        xh2s = [consts.tile([128, TW], f16, name=f"xh2_{i}")
                for i in range(2)]
        Tt = consts.tile([128, TW], f16)
        nc.vector.memset(xh2s[0][:], 127.0)
        nc.vector.memset(xh2s[1][:], 127.0)
        nc.vector.memset(Tt[:], 127.0)

        xpool = ctx.enter_context(tc.tile_pool(name="xp", bufs=2))
        tpool = ctx.enter_context(tc.tile_pool(name="tp", bufs=2))
        ypool = ctx.enter_context(tc.tile_pool(name="yp", bufs=2))
        p1pool = ctx.enter_context(tc.tile_pool(name="p1", bufs=2))
        upool = ctx.enter_context(tc.tile_pool(name="up", bufs=2))
        apool = ctx.enter_context(tc.tile_pool(name="ap", bufs=2))
        aTpool = ctx.enter_context(tc.tile_pool(name="aT", bufs=2))
        spool = ctx.enter_context(tc.tile_pool(name="sp", bufs=2))
        po = ctx.enter_context(tc.tile_pool(name="po", bufs=2, space="PSUM"))
        ptr = ctx.enter_context(tc.tile_pool(name="ptr", bufs=2,
                                             space="PSUM"))

        xv_dram = x.rearrange("(g a p) f -> g p a f", p=128, a=G)

        def pview(t, off, w):
            # [p, G, w] view of a padded tile at element offset LEAD+off
            return (t[:, LEAD + off:LEAD + off + G * IS]
                    .rearrange("p (g f) -> p g f", g=G)[:, :, 0:w])

        acts = {}

        def quant(g, halves=1):
            xt = xpool.tile([128, G * 576], f32, name="xt")
            t16 = tpool.tile([128, G * 576], f16, name="t16")
            xh2 = xh2s[g % 2]
            xdat = (pview(xh2, 0, 600)
                    .rearrange("p g (r c) -> p g r c", r=24)[:, :, :, 0:24])
            xtv = xt[:].rearrange("p (a f) -> p a f", a=G)
            t16v = t16[:].rearrange("p (a f) -> p a f", a=G)
            t16q = t16[:].rearrange("p (g r c) -> p g r c", g=G, r=24)
            h = G // halves
            for i in range(halves):
                s = slice(i * h, (i + 1) * h)
                nc.sync.dma_start(xtv[:, s], xv_dram[g][:, s])
                nc.scalar.activation(t16v[:, s], xtv[:, s], Act.Copy,
                                     bias=1408.0, scale=AF)
                nc.scalar.activation(xdat[:, s], t16q[:, s],
                                     Act.Relu, bias=1663.0, scale=-1.0)

        def core(g):
            xh2 = xh2s[g % 2]
            yt = ypool.tile([128, TW], f16, name="yt")
            yv = pview(yt, 0, 600)
            if use_T:
                # T(rr,c) = sum_dc sg0[dc]*X(rr,c+dc) on the full 625-run
                # (incl. pad row), then y' = T(r-1) + T(r+1) + row-0 taps.
                tv = pview(Tt, 0, 625)
                hh = [(dc, sg[0][dc + 1]) for dc in (-1, 0, 1)
                      if sg[0][dc + 1] != 0]
                (dc0, s0), rest = hh[0], hh[1:]
                if len(hh) >= 2 and s0 > 0:
                    dc1, s1 = rest[0]
                    nc.vector.tensor_tensor(
                        tv, pview(xh2, dc0, 625), pview(xh2, dc1, 625),
                        Alu.add if s1 > 0 else Alu.subtract)
                    rest = rest[1:]
                else:
                    nc.vector.scalar_tensor_tensor(
                        tv, pview(xh2, dc0, 625), s0,
                        pview(xh2, rest[0][0], 625), Alu.mult,
                        Alu.add if rest[0][1] > 0 else Alu.subtract)
                    rest = rest[1:]
                for dc, s in rest:
                    nc.vector.tensor_tensor(
                        tv, tv, pview(xh2, dc, 625),
                        Alu.add if s > 0 else Alu.subtract)
                nc.vector.tensor_tensor(yv, pview(Tt, -25, 600),
                                        pview(Tt, 25, 600), Alu.add)
                mid = [(dc, sg[1][dc + 1]) for dc in (-1, 0, 1)
                       if sg[1][dc + 1] != 0]
                for dc, s in mid:
                    nc.vector.tensor_tensor(
                        yv, yv, pview(xh2, dc, 600),
                        Alu.add if s > 0 else Alu.subtract)
            else:
                order = [(dr, dc, sg[dr + 1][dc + 1])
                         for dr in (-1, 0, 1) for dc in (-1, 0, 1)
                         if sg[dr + 1][dc + 1] != 0]
                order.sort(key=lambda t: -t[2])
                (dr0, dc0, s0), (dr1, dc1, s1) = order[0], order[1]
                if s0 > 0:
                    nc.vector.tensor_tensor(
                        yv, pview(xh2, 25 * dr0 + dc0, 600),
                        pview(xh2, 25 * dr1 + dc1, 600),
                        Alu.add if s1 > 0 else Alu.subtract)
                else:
                    nc.vector.scalar_tensor_tensor(
                        yv, pview(xh2, 25 * dr0 + dc0, 600), s0,
                        pview(xh2, 25 * dr1 + dc1, 600), Alu.mult,
                        Alu.add if s1 > 0 else Alu.subtract)
                for dr, dc, s in order[2:]:
                    nc.vector.tensor_tensor(
                        yv, yv, pview(xh2, 25 * dr + dc, 600),
                        Alu.add if s > 0 else Alu.subtract)

            # maxpool 2x2: vertical pairs then horizontal pairs
            yq = (pview(yt, 0, 600)
                  .rearrange("p g (rp t c) -> p g rp t c",
                             t=2, c=25)[:, :, :, :, 0:24])
            p1 = p1pool.tile([128, G * 288], f16, name="p1t")
            p1v = p1[:].rearrange("p (g rp c) -> p g rp c", g=G, rp=12)
            nc.vector.tensor_tensor(p1v, yq[:, :, :, 0, :],
                                    yq[:, :, :, 1, :], Alu.max)
            p1q = p1[:].rearrange("p (g rp c t) -> p g rp c t", g=G, rp=12,
                                  t=2)
            u = upool.tile([128, G * 144], f16, name="ut")
            uv = u[:].rearrange("p (g rp c) -> p g rp c", g=G, rp=12)
            nc.vector.tensor_tensor(uv, p1q[:, :, :, :, 0],
                                    p1q[:, :, :, :, 1], Alu.max)
            # u = round(128*P) + 1536 (f16 write rounds); clamp [1536, 1663]
            nc.vector.tensor_scalar(u[:], u[:], 0.5, S4_BIAS,
                                    Alu.mult, Alu.add)
            act = apool.tile([128, G * 144], f16, name="actt")
            nc.vector.tensor_scalar(act[:], u[:], 1536.0, 1663.0,
                                    Alu.max, Alu.min)
            # transpose now (PE identity-matmul, PSUM, scalar copy-back) so
            # fc(g)'s matmuls find aTf done.  act is [128, 9*128] flat; 9
            # non-overlapping chunk transposes; per image 2 matmuls w/ lo/hi
            # zero-padded weights (144 feats span 2 chunks).
            aTf = aTpool.tile([128, G * 144], f16, name="aTf")
            for c in range(9):
                tp = ptr.tile([128, 128], f16, name="tpsum")
                nc.tensor.transpose(tp[:], act[:, c * 128:(c + 1) * 128],
                                    ident[:])
                nc.scalar.activation(aTf[:, c * 128:(c + 1) * 128], tp[:],
                                     Act.Copy)
            acts[g] = aTf

        def fc(g):
            aTf = acts.pop(g)
            pOT = po.tile([10, G * 128], f32, name="pOTt")
            for a in range(G):
                c1, p1 = (144 * a) // 128, (144 * a) % 128
                v1 = 0 if p1 == 0 else 1
                ob = pOT[:, a * 128:(a + 1) * 128]
                nc.tensor.matmul(
                    ob, w1[:, 10 * (2 * c1 + v1):10 * (2 * c1 + v1) + 10],
                    aTf[:, 128 * c1:128 * (c1 + 1)],
                    start=True, stop=False)
                nc.tensor.matmul(
                    ob, w1[:, 10 * (2 * c1 + 2):10 * (2 * c1 + 2) + 10],
                    aTf[:, 128 * (c1 + 1):128 * (c1 + 2)],
                    start=False, stop=True)
            soT = spool.tile([10, G * 128], f32, name="soTt")
            nc.scalar.activation(soT[:], pOT[:], Act.Identity,
                                 bias=bias[:, :], scale=1.0)
            nc.sync.dma_start(out[:, g * 1024:(g + 1) * 1024], soT[:])

        # 3-stage software pipeline: quant(g) | core(g-1) | fc(g-2), so no
        # engine's in-order queue blocks the next group's prerequisites.
        for g in range(NGROUPS + 2):
            if g < NGROUPS:
                quant(g, halves=2 if g == 0 else 1)
            if 1 <= g <= NGROUPS:
                core(g - 1)
            if g >= 2:
                fc(g - 2)

    nc.compile()
    return nc


def _prep(conv_w, fc_w):
    # replicate reference weight quantization exactly (all steps exact in f32)
    cw = np.asarray(conv_w, np.float32).reshape(3, 3)
    wq = (np.round(np.clip(cw, -0.5, 0.5) * 2.0) / 2.0).astype(np.float32)
    fw = np.asarray(fc_w, np.float32)
    wfq = (np.round(np.clip(fw, -0.5, 0.5) * 2.0) / 2.0 / 8.0).astype(np.float32)
    # FC sees act values biased by +1536; fold act/128 into W (k/2048, exact
    # fp16) and remove the bias via fcb = -1536 * sum_k W[k, o].
    Wdev = np.zeros((144, 10), np.float32)
    for i in range(12):
        for j in range(12):
            Wdev[i * 12 + j, :] = wfq[:, (i + 1) * 14 + (j + 1)] / 128.0
    fcb = (-1536.0 * Wdev.sum(axis=0, dtype=np.float64)).astype(
        np.float32).reshape(10, 1)
    # flat-col layout: Wflat[144a + k] = Wdev[k]; per chunk c the rows split
    # between two images -> lo/hi zero-padded variants [128, 9*2*10]
    Wflat = np.tile(Wdev, (8, 1))  # [1152, 10]
    Wpack = np.zeros((128, 180), np.float32)
    for c in range(9):
        ac = (128 * c) // 144
        for p in range(128):
            f = 128 * c + p
            v = 0 if f // 144 == ac else 1
            Wpack[p, 10 * (2 * c + v):10 * (2 * c + v) + 10] = Wflat[f]
    return (tuple(float(v) for v in wq.flatten()),
            Wpack.astype(np.float16), fcb)


def _get_program(wq9):
    nc = _cache.get(wq9)
    if nc is None:
        nc = _build(wq9)
        _cache[wq9] = nc
    return nc


_IDENT = np.eye(128, dtype=np.float16)


def _make_in_maps(x2d, Wdev, fcb):
    return [{"x": np.ascontiguousarray(x2d[c * NPC:(c + 1) * NPC]),
             "wfc": Wdev, "fcb": fcb, "idn": _IDENT} for c in range(NCORES)]


def run(x, conv_w, fc_w, trace=False, **kw):
    from concourse.bass_utils import run_bass_kernel_spmd

    x2d = np.ascontiguousarray(
        np.asarray(x, np.float32).reshape(B, 576))
    wq9, Wdev, fcb = _prep(conv_w, fc_w)
    nc = _get_program(wq9)
    res = run_bass_kernel_spmd(nc, _make_in_maps(x2d, Wdev, fcb),
                               core_ids=list(range(NCORES)),
                               trace=trace, **kw)
    out = np.concatenate([np.asarray(r["out"]).T for r in res.results], axis=0)
    return np.ascontiguousarray(out.astype(np.float32)), res


def kernel(x, conv_w, fc_w):
    out, _ = run(x, conv_w, fc_w, trace=False)
    return out


# revision 37
# speedup vs baseline: 8.2869x; 1.0124x over previous
import sys

if "/opt/trn_rl_repo" not in sys.path:
    sys.path.insert(0, "/opt/trn_rl_repo")

import numpy as np

NCORES = 8
B = 65536
NPC = B // NCORES  # 8192 images per core
G = 8              # image-tiles (of 128) per group
NGROUPS = NPC // (128 * G)
AF = 128.0 / 127.5
IS = 626           # image block: 25 rows x 25 cols + 1 spare (even stride)
LEAD = 32          # leading pad cells (>= 26 so dr=-1,dc=-1 reads stay in-tile)
TW = LEAD + G * IS + 32  # f16 cells per partition for padded tiles

_cache = {}


def _build(wq9):
    """wq9: tuple of 9 floats, quantized conv taps in {0,+-0.5}, row-major.

    Pipeline per group (128 partitions x G images):
      scalar: t = f16(AF*x + 1408)   [= A + 1536; f16 convert rounds to int]
              xh2 = Relu(1663 - t)   [= 127 - min(A,127), upper clamp exact;
                                      A=-128 (0.2% of pixels) maps to 255]
      DVE:    y' = sum_i sigma_i*shift(xh2), sigma=-sign(w), on a 25x25-padded
              layout (pads hold 127 == "A=0").  When conv rows -1/+1 share a
              sign pattern (true for this seed) a shared horizontal term T
              cuts this to 5 tensor ops; else n_taps-1 accumulation ops.
              Then 2x2 maxpool (2 ops) + final clamp; all exact ints in f16.
      scalar: u = f16(0.5*pool + (1536 - 63.5*C))  [= round(128*P) + 1536]
      PE:     FC as K=128 + K=16 matmuls per 512-batch; the +1536 act bias is
              removed via a per-partition bias AP in the PSUM->SBUF copy.
    """
    from contextlib import ExitStack

    import concourse.tile as tile
    from concourse import bacc, mybir

    f32 = mybir.dt.float32
    f16 = mybir.dt.float16
    Alu = mybir.AluOpType
    Act = mybir.ActivationFunctionType

    nc = bacc.Bacc("TRN2", target_bir_lowering=False, debug=False,
                   num_devices=NCORES)

    # non-Copy activation biases need pre-registered const APs
    for cval in (1663.0,):
        ctensor = nc.alloc_sbuf_tensor(f"constb-{cval}", [128, 1], f32)
        nc.gpsimd.memset(ctensor.ap(), cval)
        nc.const_aps.aps[(f32, cval)] = ctensor.ap()
    nc.all_engine_barrier()

    x = nc.dram_tensor("x", [NPC, 576], f32, kind="ExternalInput").ap()
    wfc = nc.dram_tensor("wfc", [128, 180], f16, kind="ExternalInput").ap()
    fcb = nc.dram_tensor("fcb", [10, 1], f32, kind="ExternalInput").ap()
    idn = nc.dram_tensor("idn", [128, 128], f16, kind="ExternalInput").ap()
    out = nc.dram_tensor("out", [10, NPC], f32, kind="ExternalOutput").ap()

    # taps: (dr, dc, sigma) with sigma = -sign(w)
    sg = [[(-1.0 if wq9[(dr + 1) * 3 + (dc + 1)] > 0 else
            (1.0 if wq9[(dr + 1) * 3 + (dc + 1)] < 0 else 0.0))
           for dc in (-1, 0, 1)] for dr in (-1, 0, 1)]
    Cp = sum(s for row in sg for s in row)
    S4_BIAS = 1536.0 - 63.5 * Cp
    use_T = (sg[0] == sg[2] and any(s != 0 for s in sg[0]))

    with tile.TileContext(nc) as tc, ExitStack() as ctx:
        consts = ctx.enter_context(tc.tile_pool(name="consts", bufs=1))
        # Per 128-chunk lo/hi zero-padded weight blocks:
        # w1[p, 10*(2c+v)+o] = Wflat[128c+p, o] if row belongs to the lo(v=0)
        # / hi(v=1) image of chunk c, else 0.
        w1 = consts.tile([128, 180], f16)
        bias = consts.tile([10, 1], f32)
        ident = consts.tile([128, 128], f16)
        nc.sync.dma_start(w1[:], wfc[:, :])
        nc.sync.dma_start(bias[:], fcb[:, :])
        nc.sync.dma_start(ident[:], idn[:, :])
        # persistent padded buffers; pads hold 127.0 forever
        xh2s = [consts.tile([128, TW], f16, name=f"xh2_{i}")
                for i in range(2)]
        Tt = consts.tile([128, TW], f16)
        nc.vector.memset(xh2s[0][:], 127.0)
        nc.vector.memset(xh2s[1][:], 127.0)
        nc.vector.memset(Tt[:], 127.0)

        xpool = ctx.enter_context(tc.tile_pool(name="xp", bufs=2))
        tpool = ctx.enter_context(tc.tile_pool(name="tp", bufs=2))
        ypool = ctx.enter_context(tc.tile_pool(name="yp", bufs=2))
        p1pool = ctx.enter_context(tc.tile_pool(name="p1", bufs=2))
        upool = ctx.enter_context(tc.tile_pool(name="up", bufs=2))
        apool = ctx.enter_context(tc.tile_pool(name="ap", bufs=2))
        aTpool = ctx.enter_context(tc.tile_pool(name="aT", bufs=2))
        spool = ctx.enter_context(tc.tile_pool(name="sp", bufs=2))
        po = ctx.enter_context(tc.tile_pool(name="po", bufs=2, space="PSUM"))
        ptr = ctx.enter_context(tc.tile_pool(name="ptr", bufs=2,
                                             space="PSUM"))

        xv_dram = x.rearrange("(g a p) f -> g p a f", p=128, a=G)

        def pview(t, off, w):
            # [p, G, w] view of a padded tile at element offset LEAD+off
            return (t[:, LEAD + off:LEAD + off + G * IS]
                    .rearrange("p (g f) -> p g f", g=G)[:, :, 0:w])

        acts = {}

        def quant(g, halves=1):
            xt = xpool.tile([128, G * 576], f32, name="xt")
            t16 = tpool.tile([128, G * 576], f16, name="t16")
            xh2 = xh2s[g % 2]
            xdat = (pview(xh2, 0, 600)
                    .rearrange("p g (r c) -> p g r c", r=24)[:, :, :, 0:24])
            xtv = xt[:].rearrange("p (a f) -> p a f", a=G)
            t16v = t16[:].rearrange("p (a f) -> p a f", a=G)
            t16q = t16[:].rearrange("p (g r c) -> p g r c", g=G, r=24)
            h = G // halves
            for i in range(halves):
                s = slice(i * h, (i + 1) * h)
                nc.sync.dma_start(xtv[:, s], xv_dram[g][:, s])
                nc.scalar.activation(t16v[:, s], xtv[:, s], Act.Copy,
                                     bias=1408.0, scale=AF)
                nc.scalar.activation(xdat[:, s], t16q[:, s],
                                     Act.Relu, bias=1663.0, scale=-1.0)

        def core(g):
            xh2 = xh2s[g % 2]
            yt = ypool.tile([128, TW], f16, name="yt")
            yv = pview(yt, 0, 600)
            if use_T:
                # T(rr,c) = sum_dc sg0[dc]*X(rr,c+dc) on data rows only; the
                # pad row of Tt keeps its one-time 127 memset (exactly what
                # the taps there would produce).  y' = T(r-1)+T(r+1)+row-0.
                tv = pview(Tt, 0, 600)
                hh = [(dc, sg[0][dc + 1]) for dc in (-1, 0, 1)
                      if sg[0][dc + 1] != 0]
                (dc0, s0), rest = hh[0], hh[1:]
                if len(hh) >= 2 and s0 > 0:
                    dc1, s1 = rest[0]
                    nc.vector.tensor_tensor(
                        tv, pview(xh2, dc0, 600), pview(xh2, dc1, 600),
                        Alu.add if s1 > 0 else Alu.subtract)
                    rest = rest[1:]
                else:
                    nc.vector.scalar_tensor_tensor(
                        tv, pview(xh2, dc0, 600), s0,
                        pview(xh2, rest[0][0], 600), Alu.mult,
                        Alu.add if rest[0][1] > 0 else Alu.subtract)
                    rest = rest[1:]
                for dc, s in rest:
                    nc.vector.tensor_tensor(
                        tv, tv, pview(xh2, dc, 600),
                        Alu.add if s > 0 else Alu.subtract)
                nc.vector.tensor_tensor(yv, pview(Tt, -25, 600),
                                        pview(Tt, 25, 600), Alu.add)
                mid = [(dc, sg[1][dc + 1]) for dc in (-1, 0, 1)
                       if sg[1][dc + 1] != 0]
                for dc, s in mid:
                    nc.vector.tensor_tensor(
                        yv, yv, pview(xh2, dc, 600),
                        Alu.add if s > 0 else Alu.subtract)
            else:
                order = [(dr, dc, sg[dr + 1][dc + 1])
                         for dr in (-1, 0, 1) for dc in (-1, 0, 1)
                         if sg[dr + 1][dc + 1] != 0]
                order.sort(key=lambda t: -t[2])
                (dr0, dc0, s0), (dr1, dc1, s1) = order[0], order[1]
                if s0 > 0:
                    nc.vector.tensor_tensor(
                        yv, pview(xh2, 25 * dr0 + dc0, 600),
                        pview(xh2, 25 * dr1 + dc1, 600),
                        Alu.add if s1 > 0 else Alu.subtract)
                else:
                    nc.vector.scalar_tensor_tensor(
                        yv, pview(xh2, 25 * dr0 + dc0, 600), s0,
                        pview(xh2, 25 * dr1 + dc1, 600), Alu.mult,
                        Alu.add if s1 > 0 else Alu.subtract)
                for dr, dc, s in order[2:]:
                    nc.vector.tensor_tensor(
                        yv, yv, pview(xh2, 25 * dr + dc, 600),
                        Alu.add if s > 0 else Alu.subtract)

            # maxpool 2x2: vertical pairs then horizontal pairs
            yq = (pview(yt, 0, 600)
                  .rearrange("p g (rp t c) -> p g rp t c",
                             t=2, c=25)[:, :, :, :, 0:24])
            p1 = p1pool.tile([128, G * 288], f16, name="p1t")
            p1v = p1[:].rearrange("p (g rp c) -> p g rp c", g=G, rp=12)
            nc.vector.tensor_tensor(p1v, yq[:, :, :, 0, :],
                                    yq[:, :, :, 1, :], Alu.max)
            p1q = p1[:].rearrange("p (g rp c t) -> p g rp c t", g=G, rp=12,
                                  t=2)
            u = upool.tile([128, G * 144], f16, name="ut")
            uv = u[:].rearrange("p (g rp c) -> p g rp c", g=G, rp=12)
            nc.vector.tensor_tensor(uv, p1q[:, :, :, :, 0],
                                    p1q[:, :, :, :, 1], Alu.max)
            # u = round(128*P) + 1536 (f16 write rounds); clamp [1536, 1663]
            nc.vector.tensor_scalar(u[:], u[:], 0.5, S4_BIAS,
                                    Alu.mult, Alu.add)
            act = apool.tile([128, G * 144], f16, name="actt")
            nc.vector.tensor_scalar(act[:], u[:], 1536.0, 1663.0,
                                    Alu.max, Alu.min)
            # transpose now (PE identity-matmul, PSUM, scalar copy-back) so
            # fc(g)'s matmuls find aTf done.  act is [128, 9*128] flat; 9
            # non-overlapping chunk transposes; per image 2 matmuls w/ lo/hi
            # zero-padded weights (144 feats span 2 chunks).
            aTf = aTpool.tile([128, G * 144], f16, name="aTf")
            for c in range(9):
                tp = ptr.tile([128, 128], f16, name="tpsum")
                nc.tensor.transpose(tp[:], act[:, c * 128:(c + 1) * 128],
                                    ident[:])
                nc.scalar.activation(aTf[:, c * 128:(c + 1) * 128], tp[:],
                                     Act.Copy)
            acts[g] = aTf

        def fc(g):
            aTf = acts.pop(g)
            pOT = po.tile([10, G * 128], f32, name="pOTt")
            for a in range(G):
                c1, p1 = (144 * a) // 128, (144 * a) % 128
                v1 = 0 if p1 == 0 else 1
                ob = pOT[:, a * 128:(a + 1) * 128]
                nc.tensor.matmul(
                    ob, w1[:, 10 * (2 * c1 + v1):10 * (2 * c1 + v1) + 10],
                    aTf[:, 128 * c1:128 * (c1 + 1)],
                    start=True, stop=False)
                nc.tensor.matmul(
                    ob, w1[:, 10 * (2 * c1 + 2):10 * (2 * c1 + 2) + 10],
                    aTf[:, 128 * (c1 + 1):128 * (c1 + 2)],
                    start=False, stop=True)
            soT = spool.tile([10, G * 128], f32, name="soTt")
            nc.scalar.activation(soT[:], pOT[:], Act.Identity,
                                 bias=bias[:, :], scale=1.0)
            nc.sync.dma_start(out[:, g * 1024:(g + 1) * 1024], soT[:])

        # 3-stage software pipeline: quant(g) | core(g-1) | fc(g-2), so no
        # engine's in-order queue blocks the next group's prerequisites.
        for g in range(NGROUPS + 2):
            if g < NGROUPS:
                quant(g, halves=4 if g == 0 else 1)
            if 1 <= g <= NGROUPS:
                core(g - 1)
            if g >= 2:
                fc(g - 2)

    nc.compile()
    return nc


def _prep(conv_w, fc_w):
    # replicate reference weight quantization exactly (all steps exact in f32)
    cw = np.asarray(conv_w, np.float32).reshape(3, 3)
    wq = (np.round(np.clip(cw, -0.5, 0.5) * 2.0) / 2.0).astype(np.float32)
    fw = np.asarray(fc_w, np.float32)
    wfq = (np.round(np.clip(fw, -0.5, 0.5) * 2.0) / 2.0 / 8.0).astype(np.float32)
    # FC sees act values biased by +1536; fold act/128 into W (k/2048, exact
    # fp16) and remove the bias via fcb = -1536 * sum_k W[k, o].
    Wdev = np.zeros((144, 10), np.float32)
    for i in range(12):
        for j in range(12):
            Wdev[i * 12 + j, :] = wfq[:, (i + 1) * 14 + (j + 1)] / 128.0
    fcb = (-1536.0 * Wdev.sum(axis=0, dtype=np.float64)).astype(
        np.float32).reshape(10, 1)
    # flat-col layout: Wflat[144a + k] = Wdev[k]; per chunk c the rows split
    # between two images -> lo/hi zero-padded variants [128, 9*2*10]
    Wflat = np.tile(Wdev, (8, 1))  # [1152, 10]
    Wpack = np.zeros((128, 180), np.float32)
    for c in range(9):
        ac = (128 * c) // 144
        for p in range(128):
            f = 128 * c + p
            v = 0 if f // 144 == ac else 1
            Wpack[p, 10 * (2 * c + v):10 * (2 * c + v) + 10] = Wflat[f]
    return (tuple(float(v) for v in wq.flatten()),
            Wpack.astype(np.float16), fcb)


def _get_program(wq9):
    nc = _cache.get(wq9)
    if nc is None:
        nc = _build(wq9)
        _cache[wq9] = nc
    return nc


_IDENT = np.eye(128, dtype=np.float16)


def _make_in_maps(x2d, Wdev, fcb):
    return [{"x": np.ascontiguousarray(x2d[c * NPC:(c + 1) * NPC]),
             "wfc": Wdev, "fcb": fcb, "idn": _IDENT} for c in range(NCORES)]


def run(x, conv_w, fc_w, trace=False, **kw):
    from concourse.bass_utils import run_bass_kernel_spmd

    x2d = np.ascontiguousarray(
        np.asarray(x, np.float32).reshape(B, 576))
    wq9, Wdev, fcb = _prep(conv_w, fc_w)
    nc = _get_program(wq9)
    res = run_bass_kernel_spmd(nc, _make_in_maps(x2d, Wdev, fcb),
                               core_ids=list(range(NCORES)),
                               trace=trace, **kw)
    out = np.concatenate([np.asarray(r["out"]).T for r in res.results], axis=0)
    return np.ascontiguousarray(out.astype(np.float32)), res


def kernel(x, conv_w, fc_w):
    out, _ = run(x, conv_w, fc_w, trace=False)
    return out


# revision 40
# speedup vs baseline: 8.4856x; 1.0240x over previous
import sys

if "/opt/trn_rl_repo" not in sys.path:
    sys.path.insert(0, "/opt/trn_rl_repo")

import numpy as np

NCORES = 8
B = 65536
NPC = B // NCORES  # 8192 images per core
G = 8              # image-tiles (of 128) per group
NGROUPS = NPC // (128 * G)
AF = 128.0 / 127.5
IS = 626           # image block: 25 rows x 25 cols + 1 spare (even stride)
LEAD = 32          # leading pad cells (>= 26 so dr=-1,dc=-1 reads stay in-tile)
TW = LEAD + G * IS + 32  # f16 cells per partition for padded tiles

_cache = {}


def _build(wq9):
    """wq9: tuple of 9 floats, quantized conv taps in {0,+-0.5}, row-major.

    Pipeline per group (128 partitions x G images):
      scalar: t = f16(AF*x + 1408)   [= A + 1536; f16 convert rounds to int]
              xh2 = Relu(1663 - t)   [= 127 - min(A,127), upper clamp exact;
                                      A=-128 (0.2% of pixels) maps to 255]
      DVE:    y' = sum_i sigma_i*shift(xh2), sigma=-sign(w), on a 25x25-padded
              layout (pads hold 127 == "A=0").  When conv rows -1/+1 share a
              sign pattern (true for this seed) a shared horizontal term T
              cuts this to 5 tensor ops; else n_taps-1 accumulation ops.
              Then 2x2 maxpool (2 ops) + final clamp; all exact ints in f16.
      scalar: u = f16(0.5*pool + (1536 - 63.5*C))  [= round(128*P) + 1536]
      PE:     FC as K=128 + K=16 matmuls per 512-batch; the +1536 act bias is
              removed via a per-partition bias AP in the PSUM->SBUF copy.
    """
    from contextlib import ExitStack

    import concourse.tile as tile
    from concourse import bacc, mybir

    f32 = mybir.dt.float32
    f16 = mybir.dt.float16
    Alu = mybir.AluOpType
    Act = mybir.ActivationFunctionType

    nc = bacc.Bacc("TRN2", target_bir_lowering=False, debug=False,
                   num_devices=NCORES)

    # non-Copy activation biases need pre-registered const APs
    for cval in (1663.0,):
        ctensor = nc.alloc_sbuf_tensor(f"constb-{cval}", [128, 1], f32)
        nc.gpsimd.memset(ctensor.ap(), cval)
        nc.const_aps.aps[(f32, cval)] = ctensor.ap()
    nc.all_engine_barrier()

    x = nc.dram_tensor("x", [NPC, 576], f32, kind="ExternalInput").ap()
    wfc = nc.dram_tensor("wfc", [128, 180], f16, kind="ExternalInput").ap()
    wfc4 = nc.dram_tensor("wfc4", [128, 100], f16,
                          kind="ExternalInput").ap()
    fcb = nc.dram_tensor("fcb", [10, 1], f32, kind="ExternalInput").ap()
    idn = nc.dram_tensor("idn", [128, 128], f16, kind="ExternalInput").ap()
    out = nc.dram_tensor("out", [10, NPC], f32, kind="ExternalOutput").ap()

    # taps: (dr, dc, sigma) with sigma = -sign(w)
    sg = [[(-1.0 if wq9[(dr + 1) * 3 + (dc + 1)] > 0 else
            (1.0 if wq9[(dr + 1) * 3 + (dc + 1)] < 0 else 0.0))
           for dc in (-1, 0, 1)] for dr in (-1, 0, 1)]
    Cp = sum(s for row in sg for s in row)
    S4_BIAS = 1536.0 - 63.5 * Cp
    use_T = (sg[0] == sg[2] and any(s != 0 for s in sg[0]))

    with tile.TileContext(nc) as tc, ExitStack() as ctx:
        consts = ctx.enter_context(tc.tile_pool(name="consts", bufs=1))
        # Per 128-chunk lo/hi zero-padded weight blocks:
        # w1[p, 10*(2c+v)+o] = Wflat[128c+p, o] if row belongs to the lo(v=0)
        # / hi(v=1) image of chunk c, else 0.
        w1 = consts.tile([128, 180], f16)
        w2 = consts.tile([128, 100], f16)
        bias = consts.tile([10, 1], f32)
        ident = consts.tile([128, 128], f16)
        nc.sync.dma_start(w1[:], wfc[:, :])
        nc.sync.dma_start(w2[:], wfc4[:, :])
        nc.sync.dma_start(bias[:], fcb[:, :])
        nc.sync.dma_start(ident[:], idn[:, :])
        # persistent padded buffers; pads hold 127.0 forever
        xh2s = [consts.tile([128, TW], f16, name=f"xh2_{i}")
                for i in range(2)]
        Tt = consts.tile([128, TW], f16)
        nc.vector.memset(xh2s[0][:], 127.0)
        nc.vector.memset(xh2s[1][:], 127.0)
        nc.vector.memset(Tt[:], 127.0)

        xpool = ctx.enter_context(tc.tile_pool(name="xp", bufs=2))
        tpool = ctx.enter_context(tc.tile_pool(name="tp", bufs=2))
        ypool = ctx.enter_context(tc.tile_pool(name="yp", bufs=2))
        p1pool = ctx.enter_context(tc.tile_pool(name="p1", bufs=2))
        upool = ctx.enter_context(tc.tile_pool(name="up", bufs=2))
        apool = ctx.enter_context(tc.tile_pool(name="ap", bufs=2))
        aTpool = ctx.enter_context(tc.tile_pool(name="aT", bufs=2))
        spool = ctx.enter_context(tc.tile_pool(name="sp", bufs=2))
        po = ctx.enter_context(tc.tile_pool(name="po", bufs=2, space="PSUM"))
        ptr = ctx.enter_context(tc.tile_pool(name="ptr", bufs=2,
                                             space="PSUM"))

        xr = x.rearrange("(t p) f -> p t f", p=128)
        # stages: (image-tile base, tiles in stage); the tail runs as two
        # half-stages so the final FC drain is half as long.
        STAGES = [(8 * i, 8) for i in range(7)] + [(56, 4), (60, 4)]

        def pview(t, off, w, gl=G):
            # [p, gl, w] view of a padded tile at element offset LEAD+off
            return (t[:, LEAD + off:LEAD + off + gl * IS]
                    .rearrange("p (g f) -> p g f", g=gl)[:, :, 0:w])

        acts = {}

        def quant(st, halves=1):
            b0, gl = STAGES[st]
            xt = xpool.tile([128, gl * 576], f32, name="xt")
            t16 = tpool.tile([128, gl * 576], f16, name="t16")
            xh2 = xh2s[st % 2]
            xdat = (pview(xh2, 0, 600, gl)
                    .rearrange("p g (r c) -> p g r c", r=24)[:, :, :, 0:24])
            xtv = xt[:].rearrange("p (a f) -> p a f", a=gl)
            t16v = t16[:].rearrange("p (a f) -> p a f", a=gl)
            t16q = t16[:].rearrange("p (g r c) -> p g r c", g=gl, r=24)
            h = gl // halves
            for i in range(halves):
                s = slice(i * h, (i + 1) * h)
                nc.sync.dma_start(xtv[:, s], xr[:, b0 + i * h:b0 + (i + 1) * h])
                nc.scalar.activation(t16v[:, s], xtv[:, s], Act.Copy,
                                     bias=1408.0, scale=AF)
                nc.scalar.activation(xdat[:, s], t16q[:, s],
                                     Act.Relu, bias=1663.0, scale=-1.0)

        def core(st):
            b0, gl = STAGES[st]
            xh2 = xh2s[st % 2]
            yt = ypool.tile([128, TW], f16, name="yt")
            yv = pview(yt, 0, 600, gl)
            if use_T:
                # T(rr,c) = sum_dc sg0[dc]*X(rr,c+dc) on data rows only; the
                # pad row of Tt keeps its one-time 127 memset (exactly what
                # the taps there would produce).  y' = T(r-1)+T(r+1)+row-0.
                tv = pview(Tt, 0, 600, gl)
                hh = [(dc, sg[0][dc + 1]) for dc in (-1, 0, 1)
                      if sg[0][dc + 1] != 0]
                (dc0, s0), rest = hh[0], hh[1:]
                if len(hh) >= 2 and s0 > 0:
                    dc1, s1 = rest[0]
                    nc.vector.tensor_tensor(
                        tv, pview(xh2, dc0, 600, gl),
                        pview(xh2, dc1, 600, gl),
                        Alu.add if s1 > 0 else Alu.subtract)
                    rest = rest[1:]
                else:
                    nc.vector.scalar_tensor_tensor(
                        tv, pview(xh2, dc0, 600, gl), s0,
                        pview(xh2, rest[0][0], 600, gl), Alu.mult,
                        Alu.add if rest[0][1] > 0 else Alu.subtract)
                    rest = rest[1:]
                for dc, s in rest:
                    nc.vector.tensor_tensor(
                        tv, tv, pview(xh2, dc, 600, gl),
                        Alu.add if s > 0 else Alu.subtract)
                nc.vector.tensor_tensor(yv, pview(Tt, -25, 600, gl),
                                        pview(Tt, 25, 600, gl), Alu.add)
                mid = [(dc, sg[1][dc + 1]) for dc in (-1, 0, 1)
                       if sg[1][dc + 1] != 0]
                for dc, s in mid:
                    nc.vector.tensor_tensor(
                        yv, yv, pview(xh2, dc, 600, gl),
                        Alu.add if s > 0 else Alu.subtract)
            else:
                order = [(dr, dc, sg[dr + 1][dc + 1])
                         for dr in (-1, 0, 1) for dc in (-1, 0, 1)
                         if sg[dr + 1][dc + 1] != 0]
                order.sort(key=lambda t: -t[2])
                (dr0, dc0, s0), (dr1, dc1, s1) = order[0], order[1]
                if s0 > 0:
                    nc.vector.tensor_tensor(
                        yv, pview(xh2, 25 * dr0 + dc0, 600, gl),
                        pview(xh2, 25 * dr1 + dc1, 600, gl),
                        Alu.add if s1 > 0 else Alu.subtract)
                else:
                    nc.vector.scalar_tensor_tensor(
                        yv, pview(xh2, 25 * dr0 + dc0, 600, gl), s0,
                        pview(xh2, 25 * dr1 + dc1, 600, gl), Alu.mult,
                        Alu.add if s1 > 0 else Alu.subtract)
                for dr, dc, s in order[2:]:
                    nc.vector.tensor_tensor(
                        yv, yv, pview(xh2, 25 * dr + dc, 600, gl),
                        Alu.add if s > 0 else Alu.subtract)

            # maxpool 2x2: vertical pairs then horizontal pairs
            yq = (pview(yt, 0, 600, gl)
                  .rearrange("p g (rp t c) -> p g rp t c",
                             t=2, c=25)[:, :, :, :, 0:24])
            p1 = p1pool.tile([128, gl * 288], f16, name="p1t")
            p1v = p1[:].rearrange("p (g rp c) -> p g rp c", g=gl, rp=12)
            nc.vector.tensor_tensor(p1v, yq[:, :, :, 0, :],
                                    yq[:, :, :, 1, :], Alu.max)
            p1q = p1[:].rearrange("p (g rp c t) -> p g rp c t", g=gl, rp=12,
                                  t=2)
            nch = (gl * 144 + 127) // 128
            u = upool.tile([128, gl * 144], f16, name="ut")
            uv = u[:].rearrange("p (g rp c) -> p g rp c", g=gl, rp=12)
            nc.vector.tensor_tensor(uv, p1q[:, :, :, :, 0],
                                    p1q[:, :, :, :, 1], Alu.max)
            # u = round(128*P) + 1536 (f16 write rounds); clamp [1536, 1663]
            nc.vector.tensor_scalar(u[:], u[:], 0.5, S4_BIAS,
                                    Alu.mult, Alu.add)
            act = apool.tile([128, nch * 128], f16, name="actt")
            nc.vector.tensor_scalar(act[:, 0:gl * 144], u[:], 1536.0, 1663.0,
                                    Alu.max, Alu.min)
            # transpose now (PE identity-matmul, PSUM, scalar copy-back) so
            # fc(g)'s matmuls find aTf done.  act is [128, 9*128] flat; 9
            # non-overlapping chunk transposes; per image 2 matmuls w/ lo/hi
            # zero-padded weights (144 feats span 2 chunks).
            aTf = aTpool.tile([128, nch * 128], f16, name="aTf")
            for c in range(nch):
                tp = ptr.tile([128, 128], f16, name="tpsum")
                nc.tensor.transpose(tp[:], act[:, c * 128:(c + 1) * 128],
                                    ident[:])
                nc.scalar.activation(aTf[:, c * 128:(c + 1) * 128], tp[:],
                                     Act.Copy)
            acts[st] = aTf

        def fc(st):
            b0, gl = STAGES[st]
            aTf = acts.pop(st)
            wt = w1 if gl == 8 else w2
            pOT = po.tile([10, gl * 128], f32, name="pOTt")
            for a in range(gl):
                c1, p1 = (144 * a) // 128, (144 * a) % 128
                v1 = 0 if p1 == 0 else 1
                ob = pOT[:, a * 128:(a + 1) * 128]
                nc.tensor.matmul(
                    ob, wt[:, 10 * (2 * c1 + v1):10 * (2 * c1 + v1) + 10],
                    aTf[:, 128 * c1:128 * (c1 + 1)],
                    start=True, stop=False)
                nc.tensor.matmul(
                    ob, wt[:, 10 * (2 * c1 + 2):10 * (2 * c1 + 2) + 10],
                    aTf[:, 128 * (c1 + 1):128 * (c1 + 2)],
                    start=False, stop=True)
            soT = spool.tile([10, gl * 128], f32, name="soTt")
            nc.scalar.activation(soT[:], pOT[:], Act.Identity,
                                 bias=bias[:, :], scale=1.0)
            nc.sync.dma_start(out[:, b0 * 128:(b0 + gl) * 128], soT[:])

        # 3-stage software pipeline: quant(s) | core(s-1) | fc(s-2), so no
        # engine's in-order queue blocks the next stage's prerequisites.
        NS = len(STAGES)
        for s in range(NS + 2):
            if s < NS:
                quant(s, halves=4 if s == 0 else 1)
            if 1 <= s <= NS:
                core(s - 1)
            if s >= 2:
                fc(s - 2)

    nc.compile()
    return nc


def _prep(conv_w, fc_w):
    # replicate reference weight quantization exactly (all steps exact in f32)
    cw = np.asarray(conv_w, np.float32).reshape(3, 3)
    wq = (np.round(np.clip(cw, -0.5, 0.5) * 2.0) / 2.0).astype(np.float32)
    fw = np.asarray(fc_w, np.float32)
    wfq = (np.round(np.clip(fw, -0.5, 0.5) * 2.0) / 2.0 / 8.0).astype(np.float32)
    # FC sees act values biased by +1536; fold act/128 into W (k/2048, exact
    # fp16) and remove the bias via fcb = -1536 * sum_k W[k, o].
    Wdev = np.zeros((144, 10), np.float32)
    for i in range(12):
        for j in range(12):
            Wdev[i * 12 + j, :] = wfq[:, (i + 1) * 14 + (j + 1)] / 128.0
    fcb = (-1536.0 * Wdev.sum(axis=0, dtype=np.float64)).astype(
        np.float32).reshape(10, 1)
    # flat-col layout: Wflat[144a + k] = Wdev[k]; per chunk c the rows split
    # between two images -> lo/hi zero-padded variants [128, nch*2*10]
    def pack(nimg):
        nch = (nimg * 144 + 127) // 128
        Wflat = np.zeros((nch * 128, 10), np.float32)
        Wflat[0:nimg * 144] = np.tile(Wdev, (nimg, 1))
        Wp = np.zeros((128, nch * 20), np.float32)
        for c in range(nch):
            ac = (128 * c) // 144
            for p in range(128):
                f = 128 * c + p
                v = 0 if f // 144 == ac else 1
                Wp[p, 10 * (2 * c + v):10 * (2 * c + v) + 10] = Wflat[f]
        return Wp.astype(np.float16)
    return (tuple(float(v) for v in wq.flatten()), pack(8), pack(4), fcb)


def _get_program(wq9):
    nc = _cache.get(wq9)
    if nc is None:
        nc = _build(wq9)
        _cache[wq9] = nc
    return nc


_IDENT = np.eye(128, dtype=np.float16)


def _make_in_maps(x2d, W8, W4, fcb):
    return [{"x": np.ascontiguousarray(x2d[c * NPC:(c + 1) * NPC]),
             "wfc": W8, "wfc4": W4, "fcb": fcb, "idn": _IDENT}
            for c in range(NCORES)]


def run(x, conv_w, fc_w, trace=False, **kw):
    from concourse.bass_utils import run_bass_kernel_spmd

    x2d = np.ascontiguousarray(
        np.asarray(x, np.float32).reshape(B, 576))
    wq9, W8, W4, fcb = _prep(conv_w, fc_w)
    nc = _get_program(wq9)
    res = run_bass_kernel_spmd(nc, _make_in_maps(x2d, W8, W4, fcb),
                               core_ids=list(range(NCORES)),
                               trace=trace, **kw)
    out = np.concatenate([np.asarray(r["out"]).T for r in res.results], axis=0)
    return np.ascontiguousarray(out.astype(np.float32)), res


def kernel(x, conv_w, fc_w):
    out, _ = run(x, conv_w, fc_w, trace=False)
    return out
